# revision 1
# baseline (speedup 1.0000x reference)
"""Trainium2 Bass kernel for nn_Block_27848567948000 (dense transformer block).

Sharding (8 NeuronCores): 4 data-parallel groups over batch (B=4), 2-way
tensor-parallel within each pair: attention sharded over heads (5 each),
out_proj/MLP sharded over tokens (1024 each) after an 8-rank AllGather
exchange of attention outputs.

kernel(**inputs) takes FULL inputs and returns the FULL (4, 2048, 1280) output.
"""
import sys
import os

sys.path.insert(0, '/opt/trn_rl_repo')

import numpy as np
import ml_dtypes

import concourse.bass as bass
import concourse.tile as tile
from concourse import mybir, bacc
from concourse import bass_utils
from concourse.masks import make_identity

B, T, C, H, D, F = 4, 2048, 1280, 10, 128, 5120
EPS = 1e-5
N_CORES = 8
HPC = H // 2            # heads per core
CPC = HPC * D           # channels per core (640)
f32 = mybir.dt.float32
f32r = mybir.dt.float32r
bf16 = mybir.dt.bfloat16
fp8 = mybir.dt.float8e4
i32 = mybir.dt.int32
AF = mybir.ActivationFunctionType
OP = mybir.AluOpType
AX = mybir.AxisListType


def _bcast(ap, n=128):
    """Broadcast a (1, k) DRAM/SBUF AP across n partitions (step-0 partition dim)."""
    return bass.AP(tensor=ap.tensor, offset=ap.offset, ap=[[0, n]] + list(ap.ap)[1:])


def _bcast_free(ap_col, n):
    """Broadcast a (128, 1) AP along the free axis with step 0."""
    a = list(ap_col.ap)
    return bass.AP(tensor=ap_col.tensor, offset=ap_col.offset,
                   ap=[a[0], [0, n]])


def _bcast_mid(ap, reps, pos=1):
    """Insert a step-0 dim of size `reps` at position `pos` of an AP."""
    a = list(ap.ap)
    return bass.AP(tensor=ap.tensor, offset=ap.offset,
                   ap=a[:pos] + [[0, reps]] + a[pos:])




_CACHE = {}


def _get_nc(t_len=T):
    if t_len not in _CACHE:
        _CACHE[t_len] = build_nc(t_len)
    return _CACHE[t_len]


def make_in_maps(x, rotary_pos_emb, ln1_w, w_qkv, qn_w, kn_w, w_out, ln2_w,
                 w_fc1, w_fc2, t_len=T):
    """Host-side sharding prep. Returns list of per-core input dicts."""
    TL = t_len
    x = np.asarray(x, np.float32)
    rot = np.asarray(rotary_pos_emb, np.float32)
    cosd = np.cos(rot).astype(np.float32)
    sin = np.sin(rot).astype(np.float32)
    sinneg = np.concatenate([-sin[:, :64], sin[:, :64]], axis=-1).astype(np.float32)
    w_qkv_f = (np.asarray(w_qkv, np.float32)
               * np.asarray(ln1_w, np.float32)[:, None]).reshape(C, 3, H, D)
    w_fc1_f = (np.asarray(w_fc1, np.float32)
               * np.asarray(ln2_w, np.float32)[:, None])
    w_fc2_b = np.asarray(w_fc2, np.float32).astype(ml_dtypes.bfloat16)
    wo = np.asarray(w_out, np.float32).reshape(H, D, C)
    qn = np.asarray(qn_w, np.float32).reshape(1, D)
    kn = np.asarray(kn_w, np.float32).reshape(1, D)

    in_maps = []
    for c in range(N_CORES):
        b, hg = c // 2, c % 2
        heads = slice(hg * HPC, (hg + 1) * HPC)
        pheads = slice((1 - hg) * HPC, (2 - hg) * HPC)
        wq = np.ascontiguousarray(
            w_qkv_f[:, :, heads, :].reshape(C, 3 * CPC)).astype(ml_dtypes.bfloat16)
        w_outp = np.concatenate([wo[heads].reshape(CPC, C),
                                 wo[pheads].reshape(CPC, C)],
                                axis=0).astype(ml_dtypes.bfloat16)
        sel = np.zeros(16, np.float32)
        sel[0] = 1.0 - hg   # l0
        sel[1] = float(hg)  # l1
        sel[2] = float(hg)      # s0: send peer half
        sel[3] = 1.0 - hg       # s1
        peer = c ^ 1
        sel[4 + peer] = 1.0
        in_maps.append({
            'x': np.ascontiguousarray(x[b, :TL]),
            'xh': np.ascontiguousarray(x[b, hg * TL // 2:(hg + 1) * TL // 2]),
            'w_qkv': wq,
            'cosd': np.ascontiguousarray(cosd[:TL]),
            'sinneg': np.ascontiguousarray(sinneg[:TL]),
            'qn': qn, 'kn': kn,
            'selv': sel.reshape(1, 16),
            'w_out': np.ascontiguousarray(w_outp),
            'w_fc1': np.ascontiguousarray(w_fc1_f).astype(ml_dtypes.bfloat16),
            'w_fc2': np.ascontiguousarray(w_fc2_b),
        })
    return in_maps


def assemble_output(results, t_len=T):
    out = np.zeros((B, t_len, C), np.float32)
    for c in range(N_CORES):
        b, hg = c // 2, c % 2
        out[b, hg * t_len // 2:(hg + 1) * t_len // 2] = results[c]['y']
    return out


def kernel(**inputs):
    nc = _get_nc(T)
    in_maps = make_in_maps(**inputs)
    res = bass_utils.run_bass_kernel_spmd(nc, in_maps,
                                          core_ids=list(range(N_CORES)))
    return assemble_output(res.results)
def build_nc(t_len=T, n_cores=N_CORES, no_collective=False):
    """Build + compile the SPMD kernel graph for per-core sequence length t_len."""
    import contextlib
    TL = t_len
    NT = TL // 128          # token tiles (full T)
    NH = TL // 2 // 128     # token tiles of my half
    QB = TL // 512          # 512-wide query blocks
    NB = D // 32            # 32-blocks per head (4)
    NBLK = HPC * NB         # qdq blocks per tensor (20)
    inv_sqrt_d = float(1.0 / np.sqrt(D))

    nc = bacc.Bacc('TRN2', target_bir_lowering=False, debug=False,
                   num_devices=n_cores)

    # ---- DRAM I/O ----
    x_d = nc.dram_tensor('x', [TL, C], f32, kind='ExternalInput')
    xh_d = nc.dram_tensor('xh', [TL // 2, C], f32, kind='ExternalInput')
    wqkv_d = nc.dram_tensor('w_qkv', [C, 3 * CPC], bf16, kind='ExternalInput')
    cos_d = nc.dram_tensor('cosd', [TL, D], f32, kind='ExternalInput')
    sinn_d = nc.dram_tensor('sinneg', [TL, D], f32, kind='ExternalInput')
    qn_d = nc.dram_tensor('qn', [1, D], f32, kind='ExternalInput')
    kn_d = nc.dram_tensor('kn', [1, D], f32, kind='ExternalInput')
    sel_d = nc.dram_tensor('selv', [1, 16], f32, kind='ExternalInput')
    wout_d = nc.dram_tensor('w_out', [C, C], bf16, kind='ExternalInput')
    wfc1_d = nc.dram_tensor('w_fc1', [C, F], bf16, kind='ExternalInput')
    wfc2_d = nc.dram_tensor('w_fc2', [F, C], bf16, kind='ExternalInput')
    y_d = nc.dram_tensor('y', [TL // 2, C], f32, kind='ExternalOutput')

    with tile.TileContext(nc) as tc:
        with contextlib.ExitStack() as ctx:
            persist = ctx.enter_context(tc.tile_pool(name='persist', bufs=1))
            dram = ctx.enter_context(tc.tile_pool(name='dram', bufs=1, space='DRAM'))

            # ---- constants (persist) ----
            ident_b = persist.tile([128, 128], bf16)
            make_identity(nc, ident_b)
            ident_f = persist.tile([128, 128], f32)
            make_identity(nc, ident_f)
            sel_sb = persist.tile([128, 16], f32)
            nc.sync.dma_start(out=sel_sb[:], in_=_bcast(sel_d.ap()))
            zero_sb = persist.tile([128, 1], f32)
            nc.vector.memset(zero_sb[:], 0.0)
            eps_sb = persist.tile([128, 1], f32)
            nc.vector.memset(eps_sb[:], EPS)

            # ---- DRAM scratch for phase-boundary tensors ----
            attn_dram = dram.tile([TL, CPC], f32)
            loc_dram = dram.tile([TL // 2, CPC], bf16)
            peer_dram = dram.tile([TL // 2, CPC], bf16)
            x2_dram = dram.tile([TL // 2, C], f32)
            bounce_in = dram.tile([TL // 2, CPC], bf16)
            ag_outA = dram.tile([n_cores * TL // 4, CPC], bf16, addr_space='Shared')
            ag_outB = dram.tile([n_cores * TL // 4, CPC], bf16, addr_space='Shared')

            # =========== Phases A+B: qT/kT/vd live across both ===============
            with tc.tile_pool(name='ab', bufs=1) as ab:
                qT = ab.tile([128, HPC, TL], bf16)
                kT = ab.tile([128, HPC, TL], bf16)
                vd_sb = ab.tile([128, NT, HPC, D + 1], bf16)

                # ---------------- Phase A ------------------------------------
                with contextlib.ExitStack() as pa:
                    a_w = pa.enter_context(tc.tile_pool(name='a_w', bufs=1))
                    a_tmp = pa.enter_context(tc.tile_pool(name='a_tmp', bufs=3))
                    a_tm2 = pa.enter_context(tc.tile_pool(name='a_tm2', bufs=2))
                    a_qdq = pa.enter_context(tc.tile_pool(name='a_qdq', bufs=4))
                    a_ps = pa.enter_context(
                        tc.tile_pool(name='a_ps', bufs=3, space='PSUM'))
                    tr_ps = pa.enter_context(
                        tc.tile_pool(name='tr_ps', bufs=2, space='PSUM'))

                    qn_sb = a_w.tile([128, D], f32)
                    nc.sync.dma_start(out=qn_sb[:], in_=_bcast(qn_d.ap()))
                    kn_sb = a_w.tile([128, D], f32)
                    nc.sync.dma_start(out=kn_sb[:], in_=_bcast(kn_d.ap()))
                    wq_sb = a_w.tile([128, 10, 3 * CPC], bf16)
                    nc.sync.dma_start(
                        out=wq_sb[:],
                        in_=wqkv_d.ap().rearrange('(j p) c -> p j c', p=128))

                    def blk_bcast(ap2):
                        # (128, NBLK) -> (128, HPC, NB, 32), 0-step last dim
                        a = list(ap2.ap)
                        st = a[1][0]
                        return bass.AP(tensor=ap2.tensor, offset=ap2.offset,
                                       ap=[a[0], [st * NB, HPC], [st, NB], [0, 32]])

                    def v4(ap3):
                        return ap3.rearrange('p h (b e) -> p h b e', e=32)

                    def qdq(eng_a, eng_b, src4, dst4, blk_tag):
                        # src4/dst4: (128, HPC, NB, 32) APs (src psum or sbuf)
                        amax = a_qdq.tile([128, NBLK], f32, tag=blk_tag + 'am')
                        eng_a.tensor_reduce(out=amax[:], in_=src4, axis=AX.X,
                                            op=OP.max, apply_absolute_value=True)
                        eng_a.tensor_scalar_max(out=amax[:], in0=amax[:],
                                                scalar1=1e-12)
                        eb = a_qdq.tile([128, NBLK], i32, tag=blk_tag + 'eb')
                        eng_a.tensor_single_scalar(out=eb[:],
                                                   in_=amax[:].bitcast(i32),
                                                   scalar=23,
                                                   op=OP.logical_shift_right)
                        f2 = a_qdq.tile([128, NBLK], i32, tag=blk_tag + 'f2')
                        eng_a.tensor_scalar(out=f2[:], in0=eb[:], scalar1=-1,
                                            scalar2=260, op0=OP.mult, op1=OP.add)
                        sc = a_qdq.tile([128, NBLK], f32, tag=blk_tag + 'sc')
                        eng_a.tensor_single_scalar(out=sc[:].bitcast(i32),
                                                   in_=f2[:], scalar=23,
                                                   op=OP.logical_shift_left)
                        eng_a.tensor_single_scalar(out=eb[:], in_=eb[:], scalar=6,
                                                   op=OP.subtract)
                        isc = a_qdq.tile([128, NBLK], f32, tag=blk_tag + 'is')
                        eng_a.tensor_single_scalar(out=isc[:].bitcast(i32),
                                                   in_=eb[:], scalar=23,
                                                   op=OP.logical_shift_left)
                        ys = a_qdq.tile([128, HPC, NB, 32], f32, tag='ys')
                        eng_b.tensor_tensor(out=ys[:], in0=src4,
                                            in1=blk_bcast(sc[:]), op=OP.mult)
                        q8 = a_qdq.tile([128, HPC, NB, 32], fp8, tag='q8')
                        eng_b.tensor_scalar(out=q8[:], in0=ys[:], scalar1=-112.0,
                                            scalar2=112.0, op0=OP.max, op1=OP.min)
                        eng_b.tensor_tensor(out=dst4, in0=q8[:],
                                            in1=blk_bcast(isc[:]), op=OP.mult)

                    pending_T = []

                    def flush_T(upto):
                        while pending_T and pending_T[0][0] <= upto:
                            _, qd_p, kd_p, t_p = pending_T.pop(0)
                            for h in range(HPC):
                                for src_, dstT in ((qd_p, qT), (kd_p, kT)):
                                    tp2 = tr_ps.tile([128, 128], bf16, tag='tp')
                                    nc.tensor.transpose(tp2[:], src_[:, h, :],
                                                        ident_b[:])
                                    nc.any.tensor_copy(
                                        out=dstT[:, h, t_p * 128:(t_p + 1) * 128],
                                        in_=tp2[:])

                    stash = {}

                    def emit_head(t):
                        xt = a_tm2.tile([128, C], f32, tag='xt')
                        nc.sync.dma_start(out=xt[:],
                                          in_=x_d[t * 128:(t + 1) * 128, :])
                        cos_t = a_tmp.tile([128, D], f32, tag='cos_t')
                        nc.sync.dma_start(out=cos_t[:],
                                          in_=cos_d[t * 128:(t + 1) * 128, :])
                        sinn_t = a_tmp.tile([128, D], f32, tag='sinn_t')
                        nc.sync.dma_start(out=sinn_t[:],
                                          in_=sinn_d[t * 128:(t + 1) * 128, :])
                        trash = a_tm2.tile([128, C], bf16, tag='trash')
                        ssq = a_tm2.tile([128, 1], f32, tag='ssq')
                        nc.scalar.activation(out=trash[:], in_=xt[:],
                                             func=AF.Square, bias=zero_sb[:],
                                             accum_out=ssq[:])
                        rstd = a_tm2.tile([128, 1], f32, tag='rstd')
                        nc.scalar.activation(out=rstd[:], in_=ssq[:], func=AF.Sqrt,
                                             scale=float(1.0 / C), bias=eps_sb[:])
                        nc.vector.reciprocal(out=rstd[:], in_=rstd[:])
                        xn = a_tm2.tile([128, C], bf16, tag='xn')
                        nc.scalar.activation(out=xn[:], in_=xt[:], func=AF.Copy,
                                             scale=rstd[:])
                        xnT = a_tm2.tile([128, 10, 128], bf16, tag='xnT')
                        for j in range(10):
                            tp = tr_ps.tile([128, 128], bf16, tag='tp')
                            nc.tensor.transpose(tp[:], xn[:, j * 128:(j + 1) * 128],
                                                ident_b[:])
                            nc.any.tensor_copy(out=xnT[:, j, :], in_=tp[:])
                        sbufs = []
                        for g in range(3):
                            ps = a_ps.tile([128, CPC], f32, tag='qkv_ps')
                            for lo, hi in ((0, 512), (512, CPC)):
                                for j in range(10):
                                    nc.tensor.matmul(
                                        ps[:, lo:hi], xnT[:, j, :],
                                        wq_sb[:, j, g * CPC + lo:g * CPC + hi],
                                        start=(j == 0), stop=(j == 9))
                            sb = a_tmp.tile([128, CPC], f32,
                                            tag='qkv_sb' + str(g))
                            nc.scalar.copy(out=sb[:], in_=ps[:])
                            sbufs.append(sb)
                        stash[t] = (sbufs, cos_t, sinn_t)

                    def emit_tail(t):
                        (q_sb, k_sb, v_sb), cos_t, sinn_t = stash.pop(t)

                        def rope(eng, src, out, tmp):
                            src3 = src.rearrange('p (h d) -> p h d', h=HPC)
                            swap = bass.AP(tensor=src3.tensor,
                                           offset=src3.offset + 64,
                                           ap=list(src3.ap)[:2] + [[-64, 2],
                                                                   [1, 64]])
                            sin4 = bass.AP(tensor=sinn_t.tensor,
                                           offset=sinn_t[:].offset,
                                           ap=[list(sinn_t[:].ap)[0], [0, HPC],
                                               [64, 2], [1, 64]])
                            eng.tensor_tensor(
                                out=tmp[:].rearrange('p h (u d) -> p h u d', u=2),
                                in0=swap, in1=sin4, op=OP.mult)
                            eng.tensor_tensor(out=out[:], in0=src3,
                                              in1=_bcast_mid(cos_t[:], HPC),
                                              op=OP.mult)
                            eng.tensor_add(out=out[:], in0=out[:], in1=tmp[:])

                        def qknorm(app_eng, r, w_sb, sq_tag):
                            sqs = a_tm2.tile([128, HPC, D], f32, tag='scr')
                            nc.scalar.activation(out=sqs[:], in_=r[:],
                                                 func=AF.Square, bias=zero_sb[:])
                            ms = a_tm2.tile([128, HPC], f32, tag=sq_tag + 'ms')
                            nc.vector.tensor_reduce(out=ms[:], in_=sqs[:],
                                                    axis=AX.X, op=OP.add)
                            nc.scalar.activation(out=ms[:], in_=ms[:], func=AF.Sqrt,
                                                 scale=float(1.0 / D),
                                                 bias=eps_sb[:])
                            nc.vector.reciprocal(out=ms[:], in_=ms[:])
                            if app_eng is nc.vector:
                                for h in range(HPC):
                                    app_eng.scalar_tensor_tensor(
                                        out=r[:, h, :], in0=r[:, h, :],
                                        scalar=ms[:, h:h + 1], in1=w_sb[:],
                                        op0=OP.mult, op1=OP.mult)
                            else:
                                for h in range(HPC):
                                    app_eng.tensor_tensor(
                                        out=r[:, h, :], in0=r[:, h, :],
                                        in1=_bcast_free(ms[:, h:h + 1], D),
                                        op=OP.mult)
                                app_eng.tensor_tensor(
                                    out=r[:], in0=r[:],
                                    in1=_bcast_mid(w_sb[:], HPC), op=OP.mult)

                        qr = a_tm2.tile([128, HPC, D], f32, tag='qr')
                        rtmp = a_tm2.tile([128, HPC, D], f32, tag='rtmp')
                        rope(nc.vector, q_sb[:], qr, rtmp)
                        qknorm(nc.vector, qr, qn_sb, 'q')
                        qd = a_qdq.tile([128, HPC, D], bf16, tag='qd')
                        qdq(nc.vector, nc.vector, v4(qr[:]), v4(qd[:]), 'q')
                        kr = a_tm2.tile([128, HPC, D], f32, tag='kr')
                        ktmp = a_tm2.tile([128, HPC, D], f32, tag='ktmp')
                        rope(nc.gpsimd, k_sb[:], kr, ktmp)
                        qknorm(nc.gpsimd, kr, kn_sb, 'k')
                        kd = a_qdq.tile([128, HPC, D], bf16, tag='kd')
                        qdq(nc.vector, nc.gpsimd, v4(kr[:]), v4(kd[:]), 'k')
                        qdq(nc.vector, nc.vector,
                            v4(v_sb[:].rearrange('p (h d) -> p h d', h=HPC)),
                            v4(vd_sb[:, t, :, 0:D]), 'v')
                        nc.vector.memset(vd_sb[:, t, :, D:D + 1], 1.0)
                        pending_T.append((t, qd, kd, t))

                    for t in range(NT):
                        emit_head(t)
                        if t >= 1:
                            emit_tail(t - 1)
                        flush_T(t - 3)
                    emit_tail(NT - 1)
                    flush_T(NT)
                # ---------------- Phase B: attention -------------------------
                with contextlib.ExitStack() as pb:
                    b_tmp = pb.enter_context(tc.tile_pool(name='b_tmp', bufs=3))
                    pT_pool = pb.enter_context(
                        tc.tile_pool(name='pT', bufs=4 * QB + 3))
                    s_ps = pb.enter_context(
                        tc.tile_pool(name='s_ps', bufs=2, space='PSUM'))
                    o_ps = pb.enter_context(
                        tc.tile_pool(name='o_ps', bufs=3, space='PSUM'))

                    for qb in range(QB):
                        for h in range(HPC):
                            nkt = 4 * qb + 4
                            pTs = []
                            for kt in range(nkt):
                                sp = s_ps.tile([128, 512], f32, tag='sp')
                                nc.tensor.matmul(
                                    sp[:], kT[:, h, kt * 128:(kt + 1) * 128],
                                    qT[:, h, qb * 512:(qb + 1) * 512],
                                    start=True, stop=True)
                                pT = pT_pool.tile([128, 512], bf16, tag='pT')
                                nc.scalar.activation(out=pT[:], in_=sp[:],
                                                     func=AF.Exp, bias=zero_sb[:],
                                                     scale=inv_sqrt_d)
                                o = kt - 4 * qb
                                if o >= 0:
                                    nc.gpsimd.affine_select(
                                        out=pT[:], in_=pT[:], compare_op=OP.is_ge,
                                        fill=0.0, base=-128 * o,
                                        pattern=[[1, 512]], channel_multiplier=-1)
                                pTs.append(pT)
                            for ql in range(4):
                                qt = qb * 4 + ql
                                op = o_ps.tile([128, D + 1], f32, tag='op')
                                for kt in range(qt + 1):
                                    nc.tensor.matmul(
                                        op[:],
                                        pTs[kt][:, ql * 128:(ql + 1) * 128],
                                        vd_sb[:, kt, h, :],
                                        start=(kt == 0), stop=(kt == qt))
                                rc = b_tmp.tile([128, 1], f32, tag='rc')
                                nc.vector.reciprocal(out=rc[:], in_=op[:, D:D + 1])
                                anorm = b_tmp.tile([128, D], f32, tag='anorm')
                                nc.vector.tensor_scalar_mul(
                                    out=anorm[:], in0=op[:, 0:D], scalar1=rc[:])
                                nc.sync.dma_start(
                                    out=attn_dram[qt * 128:(qt + 1) * 128,
                                                  h * D:(h + 1) * D],
                                    in_=anorm[:])

            # ============ Phase C: exchange + out_proj =======================
            # C1: masked local/send halves -> DRAM / AG bounce (two halves,
            # each AllGather overlaps the remaining attention work)
            NQ = NH // 2
            with tc.tile_pool(name='c1', bufs=3) as c1:
                def emit_c1(j):
                    aj = c1.tile([128, CPC], f32, tag='aj')
                    nc.sync.dma_start(out=aj[:],
                                      in_=attn_dram[j * 128:(j + 1) * 128, :])
                    ajn = c1.tile([128, CPC], f32, tag='ajn')
                    nc.sync.dma_start(
                        out=ajn[:],
                        in_=attn_dram[(j + NH) * 128:(j + NH + 1) * 128, :])
                    locj = c1.tile([128, CPC], bf16, tag='locj')
                    nc.vector.tensor_scalar_mul(out=locj[:], in0=aj[:],
                                                scalar1=sel_sb[:, 0:1])
                    nc.vector.scalar_tensor_tensor(out=locj[:], in0=ajn[:],
                                                   scalar=sel_sb[:, 1:2],
                                                   in1=locj[:],
                                                   op0=OP.mult, op1=OP.add)
                    nc.sync.dma_start(out=loc_dram[j * 128:(j + 1) * 128, :],
                                      in_=locj[:])
                    sndj = c1.tile([128, CPC], bf16, tag='sndj')
                    sndt = c1.tile([128, CPC], bf16, tag='sndt')
                    nc.gpsimd.tensor_tensor(out=sndj[:], in0=aj[:],
                                            in1=_bcast_free(sel_sb[:, 2:3], CPC),
                                            op=OP.mult)
                    nc.gpsimd.tensor_tensor(out=sndt[:], in0=ajn[:],
                                            in1=_bcast_free(sel_sb[:, 3:4], CPC),
                                            op=OP.mult)
                    nc.gpsimd.tensor_add(out=sndj[:], in0=sndj[:], in1=sndt[:])
                    nc.sync.dma_start(out=bounce_in[j * 128:(j + 1) * 128, :],
                                      in_=sndj[:])

                for j in range(NQ):
                    emit_c1(j)
                nc.gpsimd.collective_compute(
                    'AllGather', OP.bypass,
                    ins=[bounce_in[0:TL // 4, :].opt()],
                    outs=[ag_outA[:].opt()],
                    replica_groups=[list(range(n_cores))])
                for j in range(NQ, NH):
                    emit_c1(j)
                nc.gpsimd.collective_compute(
                    'AllGather', OP.bypass,
                    ins=[bounce_in[TL // 4:TL // 2, :].opt()],
                    outs=[ag_outB[:].opt()],
                    replica_groups=[list(range(n_cores))])
            # one-hot extraction of peer blocks -> peer_dram
            wo_pool_cm = tc.tile_pool(name='wo', bufs=1)
            wo_pool = wo_pool_cm.__enter__()
            wo_sb = wo_pool.tile([128, 10, C], bf16)
            nc.sync.dma_start(
                out=wo_sb[:],
                in_=wout_d.ap().rearrange('(j p) c -> p j c', p=128))
            with tc.tile_pool(name='cpe', bufs=3) as cpe, \
                 tc.tile_pool(name='cpa', bufs=1) as cpa:
                for half, ago in ((0, ag_outA), (1, ag_outB)):
                    peer = cpa.tile([128, NQ, CPC], bf16, tag='peer')
                    for r in range(n_cores):
                        blk = cpe.tile([128, NQ, CPC], bf16, tag='agblk')
                        nc.gpsimd.dma_start(
                            out=blk[:],
                            in_=ago[r * TL // 4:(r + 1) * TL // 4, :]
                            .rearrange('(j p) c -> p j c', p=128))
                        if r == 0:
                            nc.vector.tensor_scalar_mul(out=peer[:], in0=blk[:],
                                                        scalar1=sel_sb[:, 4:5])
                        else:
                            nc.vector.scalar_tensor_tensor(
                                out=peer[:], in0=blk[:],
                                scalar=sel_sb[:, 4 + r:5 + r], in1=peer[:],
                                op0=OP.mult, op1=OP.add)
                    nc.sync.dma_start(
                        out=peer_dram[half * TL // 4:(half + 1) * TL // 4, :]
                        .rearrange('(j p) c -> p j c', p=128),
                        in_=peer[:])
            # C2: out_proj (w_out fully resident, tt-outer)
            with tc.tile_pool(name='c2', bufs=2) as c2, \
                 tc.tile_pool(name='c_ps', bufs=3, space='PSUM') as c_ps, \
                 tc.tile_pool(name='ct_ps', bufs=2, space='PSUM') as ct_ps:
                for tt in range(NH):
                    lct = c2.tile([128, CPC], bf16, tag='lct')
                    nc.sync.dma_start(out=lct[:],
                                      in_=loc_dram[tt * 128:(tt + 1) * 128, :])
                    pct = c2.tile([128, CPC], bf16, tag='pct')
                    nc.sync.dma_start(out=pct[:],
                                      in_=peer_dram[tt * 128:(tt + 1) * 128, :])
                    lT = c2.tile([128, 10, 128], bf16, tag='lT')
                    for ci in range(HPC):
                        tpl = ct_ps.tile([128, 128], bf16, tag='tpl')
                        nc.tensor.transpose(tpl[:],
                                            lct[:, ci * 128:(ci + 1) * 128],
                                            ident_b[:])
                        nc.any.tensor_copy(out=lT[:, ci, :], in_=tpl[:])
                        tpp = ct_ps.tile([128, 128], bf16, tag='tpp')
                        nc.tensor.transpose(tpp[:],
                                            pct[:, ci * 128:(ci + 1) * 128],
                                            ident_b[:])
                        nc.any.tensor_copy(out=lT[:, HPC + ci, :], in_=tpp[:])
                    for lo, hi in ((0, 512), (512, 1024), (1024, C)):
                        ps = c_ps.tile([128, 512], f32, tag='oproj_ps')
                        for jj in range(10):
                            nc.tensor.matmul(
                                ps[:, 0:hi - lo],
                                lT[:, jj, :],
                                wo_sb[:, jj, lo:hi],
                                start=(jj == 0), stop=(jj == 9))
                        xht = c2.tile([128, 512], f32, tag='xht')
                        nc.sync.dma_start(
                            out=xht[:, 0:hi - lo],
                            in_=xh_d[tt * 128:(tt + 1) * 128, lo:hi])
                        x2t = c2.tile([128, 512], f32, tag='x2t')
                        nc.vector.tensor_add(out=x2t[:, 0:hi - lo],
                                             in0=ps[:, 0:hi - lo],
                                             in1=xht[:, 0:hi - lo])
                        nc.sync.dma_start(
                            out=x2_dram[tt * 128:(tt + 1) * 128, lo:hi],
                            in_=x2t[:, 0:hi - lo])
            wo_pool_cm.__exit__(None, None, None)
            # ================= Phase D: MLP ==================================
            with contextlib.ExitStack() as pd:
                d_tmp = pd.enter_context(tc.tile_pool(name='d_tmp', bufs=3))
                h2_pool = pd.enter_context(tc.tile_pool(name='h2', bufs=1))
                h2T = h2_pool.tile([128, F // 128, TL // 2], bf16)
                tchunks = [(s, min(s + 512, TL // 2))
                           for s in range(0, TL // 2, 512)]

                with tc.tile_pool(name='xn2', bufs=1) as xn2_pool, \
                     tc.tile_pool(name='dt_ps', bufs=2, space='PSUM') as dt_ps, \
                     tc.tile_pool(name='h_ps', bufs=3, space='PSUM') as h_ps, \
                     tc.tile_pool(name='wf1', bufs=3) as wf1_pool:
                    xn2T = xn2_pool.tile([128, 10, TL // 2], bf16)
                    for tt in range(NH):
                        x2t = d_tmp.tile([128, C], f32, tag='x2ld')
                        nc.sync.dma_start(out=x2t[:],
                                          in_=x2_dram[tt * 128:(tt + 1) * 128, :])
                        trash2 = d_tmp.tile([128, C], bf16, tag='trash2')
                        ssq = d_tmp.tile([128, 1], f32, tag='ssq2')
                        nc.scalar.activation(out=trash2[:], in_=x2t[:],
                                             func=AF.Square, bias=zero_sb[:],
                                             accum_out=ssq[:])
                        rstd = d_tmp.tile([128, 1], f32, tag='rstd2')
                        nc.scalar.activation(out=rstd[:], in_=ssq[:], func=AF.Sqrt,
                                             scale=float(1.0 / C), bias=eps_sb[:])
                        nc.vector.reciprocal(out=rstd[:], in_=rstd[:])
                        xn2 = d_tmp.tile([128, C], bf16, tag='xn2t')
                        nc.scalar.activation(out=xn2[:], in_=x2t[:], func=AF.Copy,
                                             scale=rstd[:])
                        for j in range(10):
                            tp = dt_ps.tile([128, 128], bf16, tag='xn2_tp')
                            nc.tensor.transpose(tp[:],
                                                xn2[:, j * 128:(j + 1) * 128],
                                                ident_b[:])
                            nc.any.tensor_copy(
                                out=xn2T[:, j, tt * 128:(tt + 1) * 128],
                                in_=tp[:])

                    for fi in range(F // 128):
                        wf1 = wf1_pool.tile([128, 10, 128], bf16, tag='wf1')
                        nc.sync.dma_start(
                            out=wf1[:],
                            in_=wfc1_d[:, fi * 128:(fi + 1) * 128]
                            .rearrange('(j p) c -> p j c', p=128))
                        for clo, chi in tchunks:
                            hps = h_ps.tile([128, 512], f32, tag='hps')
                            for j in range(10):
                                nc.tensor.matmul(
                                    hps[:, 0:chi - clo],
                                    wf1[:, j, :],
                                    xn2T[:, j, clo:chi],
                                    start=(j == 0), stop=(j == 9))
                            hrelu = d_tmp.tile([128, 512], bf16, tag='hrelu')
                            nc.scalar.activation(out=hrelu[:, 0:chi - clo],
                                                 in_=hps[:, 0:chi - clo],
                                                 func=AF.Relu, bias=zero_sb[:])
                            nc.vector.tensor_mul(out=h2T[:, fi, clo:chi],
                                                 in0=hrelu[:, 0:chi - clo],
                                                 in1=hrelu[:, 0:chi - clo])

                with tc.tile_pool(name='y_ps', bufs=NH, space='PSUM') as y_ps, \
                     tc.tile_pool(name='wf2', bufs=3) as wf2_pool:
                    for lo, hi in ((0, 512), (512, 1024), (1024, C)):
                        yps = []
                        for _i in range(NH):
                            ypt = y_ps.tile([128, 512], f32, tag='yps')
                            yps.append(ypt)
                        for fi in range(F // 128):
                            wf2 = wf2_pool.tile([128, 512], bf16, tag='wf2')
                            nc.sync.dma_start(
                                out=wf2[:, 0:hi - lo],
                                in_=wfc2_d[fi * 128:(fi + 1) * 128, lo:hi])
                            for tt in range(NH):
                                nc.tensor.matmul(
                                    yps[tt][:, 0:hi - lo],
                                    h2T[:, fi, tt * 128:(tt + 1) * 128],
                                    wf2[:, 0:hi - lo],
                                    start=(fi == 0), stop=(fi == F // 128 - 1))
                        for tt in range(NH):
                            x2s = d_tmp.tile([128, 512], f32, tag='x2s')
                            nc.sync.dma_start(
                                out=x2s[:, 0:hi - lo],
                                in_=x2_dram[tt * 128:(tt + 1) * 128, lo:hi])
                            yo = d_tmp.tile([128, 512], f32, tag='yo')
                            nc.vector.tensor_add(out=yo[:, 0:hi - lo],
                                                 in0=yps[tt][:, 0:hi - lo],
                                                 in1=x2s[:, 0:hi - lo])
                            nc.sync.dma_start(
                                out=y_d[tt * 128:(tt + 1) * 128, lo:hi],
                                in_=yo[:, 0:hi - lo])

    nc.compile()
    return nc


_CACHE = {}


def _get_nc(t_len=T):
    if t_len not in _CACHE:
        _CACHE[t_len] = build_nc(t_len)
    return _CACHE[t_len]


def make_in_maps(x, rotary_pos_emb, ln1_w, w_qkv, qn_w, kn_w, w_out, ln2_w,
                 w_fc1, w_fc2, t_len=T):
    """Host-side sharding prep. Returns list of per-core input dicts."""
    TL = t_len
    x = np.asarray(x, np.float32)
    rot = np.asarray(rotary_pos_emb, np.float32)
    cosd = np.cos(rot).astype(np.float32)
    sin = np.sin(rot).astype(np.float32)
    sinneg = np.concatenate([-sin[:, :64], sin[:, :64]], axis=-1).astype(np.float32)
    w_qkv_f = (np.asarray(w_qkv, np.float32)
               * np.asarray(ln1_w, np.float32)[:, None]).reshape(C, 3, H, D)
    w_fc1_f = (np.asarray(w_fc1, np.float32)
               * np.asarray(ln2_w, np.float32)[:, None])
    w_fc2_b = np.asarray(w_fc2, np.float32).astype(ml_dtypes.bfloat16)
    wo = np.asarray(w_out, np.float32).reshape(H, D, C)
    qn = np.asarray(qn_w, np.float32).reshape(1, D)
    kn = np.asarray(kn_w, np.float32).reshape(1, D)

    in_maps = []
    for c in range(N_CORES):
        b, hg = c // 2, c % 2
        heads = slice(hg * HPC, (hg + 1) * HPC)
        pheads = slice((1 - hg) * HPC, (2 - hg) * HPC)
        wq = np.ascontiguousarray(
            w_qkv_f[:, :, heads, :].reshape(C, 3 * CPC)).astype(ml_dtypes.bfloat16)
        w_outp = np.concatenate([wo[heads].reshape(CPC, C),
                                 wo[pheads].reshape(CPC, C)],
                                axis=0).astype(ml_dtypes.bfloat16)
        sel = np.zeros(16, np.float32)
        sel[0] = 1.0 - hg   # l0
        sel[1] = float(hg)  # l1
        sel[2] = float(hg)      # s0: send peer half
        sel[3] = 1.0 - hg       # s1
        peer = c ^ 1
        sel[4 + peer] = 1.0
        in_maps.append({
            'x': np.ascontiguousarray(x[b, :TL]),
            'xh': np.ascontiguousarray(x[b, hg * TL // 2:(hg + 1) * TL // 2]),
            'w_qkv': wq,
            'cosd': np.ascontiguousarray(cosd[:TL]),
            'sinneg': np.ascontiguousarray(sinneg[:TL]),
            'qn': qn, 'kn': kn,
            'selv': sel.reshape(1, 16),
            'w_out': np.ascontiguousarray(w_outp),
            'w_fc1': np.ascontiguousarray(w_fc1_f).astype(ml_dtypes.bfloat16),
            'w_fc2': np.ascontiguousarray(w_fc2_b),
        })
    return in_maps


def assemble_output(results, t_len=T):
    out = np.zeros((B, t_len, C), np.float32)
    for c in range(N_CORES):
        b, hg = c // 2, c % 2
        out[b, hg * t_len // 2:(hg + 1) * t_len // 2] = results[c]['y']
    return out


def kernel(**inputs):
    nc = _get_nc(T)
    in_maps = make_in_maps(**inputs)
    res = bass_utils.run_bass_kernel_spmd(nc, in_maps,
                                          core_ids=list(range(N_CORES)))
    return assemble_output(res.results)



# revision 9
# speedup vs baseline: 1.0107x; 1.0107x over previous
"""Trainium2 Bass kernel for nn_Block_27848567948000 (dense transformer block).

Sharding (8 NeuronCores): 4 data-parallel groups over batch (B=4), 2-way
tensor-parallel within each pair: attention sharded over heads (5 each).
out_proj computed as per-head partial sums over ALL T, summed + token-scattered
via a pairwise ReduceScatter; MLP over the core's T/2 token half.

kernel(**inputs) takes FULL inputs and returns the FULL (4, 2048, 1280) output.
"""
import sys

sys.path.insert(0, '/opt/trn_rl_repo')

import numpy as np
import ml_dtypes

import concourse.bass as bass
import concourse.tile as tile
from concourse import mybir, bacc
from concourse import bass_utils
from concourse.masks import make_identity

B, T, C, H, D, F = 4, 2048, 1280, 10, 128, 5120
EPS = 1e-5
N_CORES = 8
HPC = H // 2            # heads per core (5)
CPC = HPC * D           # channels per core (640)
f32 = mybir.dt.float32
bf16 = mybir.dt.bfloat16
fp8 = mybir.dt.float8e4
i32 = mybir.dt.int32
AF = mybir.ActivationFunctionType
OP = mybir.AluOpType
AX = mybir.AxisListType

NT = T // 128            # 16 token tiles
NH = T // 2 // 128       # 8 token tiles in my half
QB = T // 512            # 4 query blocks
NBLK = HPC * 4           # 20 mxfp8 blocks per tensor per token
INV_SQRT_D = float(1.0 / np.sqrt(D))
NEG = -30000.0


def _ap(t_ap, offset_delta, pattern):
    return bass.AP(tensor=t_ap.tensor, offset=t_ap.offset + offset_delta,
                   ap=pattern)


def build_nc(t_len=T, n_cores=N_CORES):
    import contextlib
    nc = bacc.Bacc('TRN2', target_bir_lowering=False, debug=False,
                   num_devices=n_cores)

    # ---- DRAM I/O ----
    x_d = nc.dram_tensor('x', [T, C], f32, kind='ExternalInput')
    xh_d = nc.dram_tensor('xh', [T // 2, C], f32, kind='ExternalInput')
    wqkv_d = nc.dram_tensor('w_qkv', [C, 3 * CPC], bf16, kind='ExternalInput')
    cosq_d = nc.dram_tensor('cosq', [T, D], bf16, kind='ExternalInput')
    sinq_d = nc.dram_tensor('sinq', [T, D], bf16, kind='ExternalInput')
    cosk_d = nc.dram_tensor('cosk', [T, D], bf16, kind='ExternalInput')
    sink_d = nc.dram_tensor('sink', [T, D], bf16, kind='ExternalInput')
    wout_d = nc.dram_tensor('w_out', [CPC, C], bf16, kind='ExternalInput')
    wfc1_d = nc.dram_tensor('w_fc1', [C, F], bf16, kind='ExternalInput')
    wfc2_d = nc.dram_tensor('w_fc2', [F, C], bf16, kind='ExternalInput')
    y_d = nc.dram_tensor('y', [T // 2, C], f32, kind='ExternalOutput')

    with tile.TileContext(nc) as tc:
        with contextlib.ExitStack() as ctx:
            persist = ctx.enter_context(tc.tile_pool(name='persist', bufs=1))
            dram = ctx.enter_context(tc.tile_pool(name='dram', bufs=1,
                                                  space='DRAM'))

            # ---- constants ----
            ident_b = persist.tile([128, 128], bf16)
            make_identity(nc, ident_b)
            ones128 = persist.tile([128, 128], bf16)
            nc.vector.memset(ones128[:], 1.0)
            zero_sb = persist.tile([128, 1], f32)
            nc.vector.memset(zero_sb[:], 0.0)
            eps_sb = persist.tile([128, 1], f32)
            nc.vector.memset(eps_sb[:], EPS)
            scr_sq = persist.tile([128, C], bf16)   # Square-output scratch

            # DRAM scratch for the collective
            rs_in = dram.tile([T, C], bf16)
            rs_out = dram.tile([T // 2, C], bf16)

            with contextlib.ExitStack() as pab:
                ab = pab.enter_context(tc.tile_pool(name='ab', bufs=1))
                qT = ab.tile([128, HPC, T], bf16)
                kT = ab.tile([128, HPC, T], bf16)
                vd_sb = ab.tile([128, NT, HPC, 130], bf16)
                attnT = ab.tile([128, HPC, T], bf16)
                nc.vector.memset(vd_sb[:, :, :, 128:129], 1.0)

                # ====== phases A+B ======
                with contextlib.ExitStack() as pin:
                    a_w = pin.enter_context(tc.tile_pool(name='a_w', bufs=1))
                    wq_sb = a_w.tile([128, 10, 3 * CPC], bf16)
                    nc.sync.dma_start(
                        out=wq_sb[:],
                        in_=wqkv_d.ap().rearrange('(j p) c -> p j c', p=128))
                    # causal staircase masks for diagonal 128x512 k-tiles
                    msk = a_w.tile([128, 4, 512], bf16)
                    nc.vector.memset(msk[:], 0.0)
                    for o in range(4):
                        nc.gpsimd.affine_select(
                            out=msk[:, o, :], in_=msk[:, o, :],
                            compare_op=OP.is_ge, fill=NEG, base=-128 * o,
                            pattern=[[1, 512]], channel_multiplier=-1)
                    cq_sb = a_w.tile([128, NT, D], bf16)
                    nc.sync.dma_start(
                        out=cq_sb[:],
                        in_=cosq_d.ap().rearrange('(t p) d -> p t d', p=128))
                    sq_sb = a_w.tile([128, NT, D], bf16)
                    nc.sync.dma_start(
                        out=sq_sb[:],
                        in_=sinq_d.ap().rearrange('(t p) d -> p t d', p=128))
                    ck_sb = a_w.tile([128, NT, D], bf16)
                    nc.sync.dma_start(
                        out=ck_sb[:],
                        in_=cosk_d.ap().rearrange('(t p) d -> p t d', p=128))
                    sk_sb = a_w.tile([128, NT, D], bf16)
                    nc.sync.dma_start(
                        out=sk_sb[:],
                        in_=sink_d.ap().rearrange('(t p) d -> p t d', p=128))

                    a_t = pin.enter_context(tc.tile_pool(name='a_t', bufs=2))
                    a_s = pin.enter_context(tc.tile_pool(name='a_s', bufs=2))
                    a_q = pin.enter_context(tc.tile_pool(name='a_q', bufs=2))
                    pT_pool = pin.enter_context(
                        tc.tile_pool(name='pT', bufs=4))
                    b_t = pin.enter_context(tc.tile_pool(name='b_t', bufs=2))
                    ps512 = pin.enter_context(
                        tc.tile_pool(name='ps512', bufs=3, space='PSUM'))
                    ops_ps = pin.enter_context(
                        tc.tile_pool(name='ops_ps', bufs=1, space='PSUM'))
                    psT = pin.enter_context(
                        tc.tile_pool(name='psT', bufs=2, space='PSUM'))
                    psD = pin.enter_context(
                        tc.tile_pool(name='psD', bufs=2, space='PSUM'))

                    stash = {}

                    def emit_head(t):
                        xt = a_s.tile([128, C], f32, tag='xt')
                        nc.sync.dma_start(out=xt[:],
                                          in_=x_d[t * 128:(t + 1) * 128, :])
                        ssq = a_s.tile([128, 1], f32, tag='ssq')
                        nc.scalar.activation(out=scr_sq[:], in_=xt[:],
                                             func=AF.Square, bias=zero_sb[:],
                                             accum_out=ssq[:])
                        rstd = a_s.tile([128, 1], f32, tag='rstd')
                        nc.scalar.activation(out=rstd[:], in_=ssq[:],
                                             func=AF.Sqrt,
                                             scale=float(1.0 / C),
                                             bias=eps_sb[:])
                        nc.vector.reciprocal(out=rstd[:], in_=rstd[:])
                        xn = a_s.tile([128, C], bf16, tag='xn')
                        nc.scalar.activation(out=xn[:], in_=xt[:],
                                             func=AF.Copy, scale=rstd[:])
                        xnT = a_s.tile([128, 10, 128], bf16, tag='xnT')
                        for jg, (lo, hi) in enumerate(((0, 4), (4, 8),
                                                      (8, 10))):
                            tp = psT.tile([128, 640], bf16, tag='tp')
                            for j in range(lo, hi):
                                nc.tensor.transpose(
                                    tp[:, (j - lo) * 128:(j - lo + 1) * 128],
                                    xn[:, j * 128:(j + 1) * 128], ident_b[:])
                            nc.vector.tensor_copy(
                                out=xnT[:, lo:hi, :],
                                in_=tp[:, 0:(hi - lo) * 128].rearrange(
                                    'p (j d) -> p j d', d=128))
                        # QKV (chunk-outer, j-mid, g-inner: LDW amortized)
                        qf = a_q.tile([128, CPC], bf16, tag='qf')
                        kf = a_q.tile([128, CPC], bf16, tag='kf')
                        vf = a_q.tile([128, CPC], bf16, tag='vf')
                        dsts = (qf, kf, vf)
                        for lo, hi in ((0, 512), (512, 640)):
                            pss = [ps512.tile([128, 512], f32, tag='mm',
                                              name='qkvps')
                                   for _ in range(3)]
                            for j in range(10):
                                for g in range(3):
                                    nc.tensor.matmul(
                                        pss[g][:, 0:hi - lo], xnT[:, j, :],
                                        wq_sb[:, j,
                                              g * CPC + lo:g * CPC + hi],
                                        start=(j == 0), stop=(j == 9))
                            for g in range(3):
                                if g == 1:
                                    nc.scalar.copy(out=dsts[g][:, lo:hi],
                                                   in_=pss[g][:, 0:hi - lo])
                                else:
                                    nc.vector.tensor_copy(
                                        out=dsts[g][:, lo:hi],
                                        in_=pss[g][:, 0:hi - lo])
                        stash[t] = (qf, kf, vf)

                    def rope(eng, src, cos_t, sin_t, out):
                        # out[p,h,d] = src*cos + swap(src)*sinneg   (bf16)
                        src3 = src[:].rearrange('p (h d) -> p h d', h=HPC)
                        pa = list(src3.ap)
                        swap = _ap(src3, 64, pa[:2] + [[-64, 2], [1, 64]])
                        ca = list(cos_t.ap)
                        cos4 = _ap(cos_t, 0, [ca[0], [0, HPC], [1, 128]])
                        sin4 = _ap(sin_t, 0,
                                   [ca[0], [0, HPC], [64, 2], [1, 64]])
                        tmp = a_t.tile([128, HPC, D], bf16, tag='rtmp')
                        eng.tensor_tensor(
                            out=tmp[:].rearrange('p h (u d) -> p h u d', u=2),
                            in0=swap, in1=sin4, op=OP.mult)
                        eng.tensor_tensor(out=out[:], in0=src3, in1=cos4,
                                          op=OP.mult)
                        eng.tensor_add(out=out[:], in0=out[:], in1=tmp[:])

                    def blk4(ap20, reps=32):
                        # (128,20) -> (128,5,4,reps) block broadcast
                        a = list(ap20.ap)
                        st = a[-1][0]
                        return bass.AP(tensor=ap20.tensor, offset=ap20.offset,
                                       ap=[a[0], [4 * st, HPC], [st, 4],
                                           [0, reps]])

                    def hb(ap5, reps=4):
                        # (128,5) -> (128,5,reps) broadcast
                        a = list(ap5.ap)
                        return bass.AP(tensor=ap5.tensor, offset=ap5.offset,
                                       ap=[a[0], [a[-1][0], HPC], [0, reps]])

                    def v4(x):
                        return x.rearrange('p h (b e) -> p h b e', e=32)

                    def emit_tail(t):
                        qf, kf, vf = stash.pop(t)
                        # rms of pre-rope q/k (rope is norm-preserving)
                        msq = a_t.tile([128, 2, HPC], f32, tag='msq')
                        for i, src in enumerate((qf, kf)):
                            for h in range(HPC):
                                nc.scalar.activation(
                                    out=scr_sq[:, 0:D],
                                    in_=src[:, h * D:(h + 1) * D],
                                    func=AF.Square, bias=zero_sb[:],
                                    accum_out=msq[:, i, h:h + 1])
                        nc.scalar.activation(out=msq[:], in_=msq[:],
                                             func=AF.Sqrt,
                                             scale=float(1.0 / D),
                                             bias=eps_sb[:])
                        nc.vector.reciprocal(out=msq[:], in_=msq[:])
                        # rope (q on vector, k on gpsimd)
                        zq = a_t.tile([128, HPC, D], bf16, tag='zq')
                        rope(nc.vector, qf, cq_sb[:, t, :], sq_sb[:, t, :],
                             zq)
                        zk = a_t.tile([128, HPC, D], bf16, tag='zk')
                        rope(nc.gpsimd, kf, ck_sb[:, t, :], sk_sb[:, t, :],
                             zk)
                        # block amax; amn = amax*rstd (q,k) or amax (v)
                        amn = a_t.tile([128, 3, NBLK], f32, tag='amn')
                        nc.vector.tensor_reduce(
                            out=amn[:, 0, :], in_=v4(zq[:]), axis=AX.X,
                            op=OP.max, apply_absolute_value=True)
                        nc.vector.tensor_reduce(
                            out=amn[:, 1, :], in_=v4(zk[:]), axis=AX.X,
                            op=OP.max, apply_absolute_value=True)
                        nc.vector.tensor_reduce(
                            out=amn[:, 2, :],
                            in_=vf[:].rearrange('p (h b e) -> p h b e',
                                                h=HPC, e=32),
                            axis=AX.X, op=OP.max, apply_absolute_value=True)
                        for i in range(2):
                            nc.vector.tensor_tensor(
                                out=amn[:, i, :].rearrange(
                                    'p (h b) -> p h b', h=HPC),
                                in0=amn[:, i, :].rearrange(
                                    'p (h b) -> p h b', h=HPC),
                                in1=hb(msq[:, i, :]), op=OP.mult)
                        nc.vector.tensor_scalar_max(out=amn[:], in0=amn[:],
                                                    scalar1=1e-12)
                        eb = a_t.tile([128, 3, NBLK], i32, tag='eb')
                        nc.vector.tensor_single_scalar(
                            out=eb[:], in_=amn[:].bitcast(i32), scalar=23,
                            op=OP.logical_shift_right)
                        sc = a_t.tile([128, 3, NBLK], f32, tag='sc')
                        nc.vector.tensor_scalar(
                            out=sc[:].bitcast(i32), in0=eb[:], scalar1=-1,
                            scalar2=260, op0=OP.mult, op1=OP.add)
                        nc.vector.tensor_single_scalar(
                            out=sc[:].bitcast(i32), in_=sc[:].bitcast(i32),
                            scalar=23, op=OP.logical_shift_left)
                        isc = a_t.tile([128, 3, NBLK], f32, tag='isc')
                        nc.vector.tensor_single_scalar(
                            out=isc[:].bitcast(i32), in_=eb[:], scalar=6,
                            op=OP.subtract)
                        nc.vector.tensor_single_scalar(
                            out=isc[:].bitcast(i32), in_=isc[:].bitcast(i32),
                            scalar=23, op=OP.logical_shift_left)
                        msc = a_t.tile([128, 2, NBLK], f32, tag='msc')
                        for i in range(2):
                            nc.vector.tensor_tensor(
                                out=msc[:, i, :].rearrange(
                                    'p (h b) -> p h b', h=HPC),
                                in0=sc[:, i, :].rearrange(
                                    'p (h b) -> p h b', h=HPC),
                                in1=hb(msq[:, i, :]), op=OP.mult)
                        # quantize q (vector)
                        ys = a_t.tile([128, HPC, D], bf16, tag='ys')
                        q8 = a_t.tile([128, HPC, D], fp8, tag='q8')
                        qd = a_t.tile([128, HPC, D], bf16, tag='qd')
                        nc.vector.tensor_tensor(out=v4(ys[:]), in0=v4(zq[:]),
                                                in1=blk4(msc[:, 0, :]),
                                                op=OP.mult)
                        nc.vector.tensor_scalar(out=q8[:], in0=ys[:],
                                                scalar1=-112.0,
                                                scalar2=112.0,
                                                op0=OP.max, op1=OP.min)
                        nc.vector.tensor_tensor(out=v4(qd[:]), in0=v4(q8[:]),
                                                in1=blk4(isc[:, 0, :]),
                                                op=OP.mult)
                        # quantize k (gpsimd mults, vector fp8 cast)
                        ysk = a_t.tile([128, HPC, D], bf16, tag='ys')
                        k8 = a_t.tile([128, HPC, D], fp8, tag='q8')
                        kd = a_t.tile([128, HPC, D], bf16, tag='kd')
                        nc.gpsimd.tensor_tensor(out=v4(ysk[:]),
                                                in0=v4(zk[:]),
                                                in1=blk4(msc[:, 1, :]),
                                                op=OP.mult)
                        nc.vector.tensor_scalar(out=k8[:], in0=ysk[:],
                                                scalar1=-112.0,
                                                scalar2=112.0,
                                                op0=OP.max, op1=OP.min)
                        nc.gpsimd.tensor_tensor(out=v4(kd[:]), in0=v4(k8[:]),
                                                in1=blk4(isc[:, 1, :]),
                                                op=OP.mult)
                        # quantize v (vector; deq straight into vd_sb)
                        ysv = a_t.tile([128, HPC, D], bf16, tag='ys')
                        v8 = a_t.tile([128, HPC, D], fp8, tag='q8')
                        nc.vector.tensor_tensor(
                            out=v4(ysv[:]),
                            in0=v4(vf[:].rearrange('p (h d) -> p h d',
                                                   h=HPC)),
                            in1=blk4(sc[:, 2, :]), op=OP.mult)
                        nc.vector.tensor_scalar(out=v8[:], in0=ysv[:],
                                                scalar1=-112.0,
                                                scalar2=112.0,
                                                op0=OP.max, op1=OP.min)
                        nc.vector.tensor_tensor(
                            out=v4(vd_sb[:, t, :, 0:D]), in0=v4(v8[:]),
                            in1=blk4(isc[:, 2, :]), op=OP.mult)
                        # transpose qd/kd into qT/kT
                        for src, dstT in ((qd, qT), (kd, kT)):
                            tp = psT.tile([128, 640], bf16, tag='tp')
                            for h in range(HPC):
                                nc.tensor.transpose(
                                    tp[:, h * 128:(h + 1) * 128],
                                    src[:, h, :], ident_b[:])
                            nc.vector.tensor_copy(
                                out=dstT[:, :, t * 128:(t + 1) * 128],
                                in_=tp[:].rearrange('p (h d) -> p h d',
                                                    h=HPC))

                    def emit_attn(qb):
                        nkt = 4 * qb + 4
                        for h in range(HPC):
                            dps = psD.tile([128, 512], f32, tag='dps')
                            ops = ops_ps.tile([128, 512], f32, tag='ops')
                            for kt in range(nkt):
                                sp = ps512.tile([128, 512], f32, tag='mm')
                                o = kt - 4 * qb
                                nc.tensor.matmul(
                                    sp[:],
                                    kT[:, h, kt * 128:(kt + 1) * 128],
                                    qT[:, h, qb * 512:(qb + 1) * 512],
                                    start=True, stop=(o < 0))
                                if o >= 0:
                                    nc.tensor.matmul(sp[:], ident_b[:],
                                                     msk[:, o, :],
                                                     start=False, stop=True)
                                pT = pT_pool.tile([128, 512], bf16, tag='pT')
                                nc.scalar.activation(out=pT[:], in_=sp[:],
                                                     func=AF.Exp,
                                                     bias=zero_sb[:],
                                                     scale=INV_SQRT_D)
                                nc.tensor.matmul(dps[:], ones128[:], pT[:],
                                                 start=(kt == 0),
                                                 stop=(kt == nkt - 1))
                                nc.tensor.matmul(ops[:],
                                                 vd_sb[:, kt, h, 0:128],
                                                 pT[:],
                                                 start=(kt == 0),
                                                 stop=(kt == nkt - 1))
                            rd = b_t.tile([128, 512], f32, tag='rd')
                            nc.vector.reciprocal(out=rd[:], in_=dps[:])
                            nc.vector.tensor_tensor(
                                out=attnT[:, h, qb * 512:(qb + 1) * 512],
                                in0=ops[:], in1=rd[:], op=OP.mult)

                    # ---- interleaved A+B emission ----
                    for t in range(NT):
                        emit_head(t)
                        if t >= 1:
                            emit_tail(t - 1)
                        if t >= 4 and t % 4 == 0:
                            emit_attn(t // 4 - 1)
                    emit_tail(NT - 1)
                    emit_attn(QB - 1)

                # ====== phase C: out_proj partials + ReduceScatter ======
                with tc.tile_pool(name='c_w', bufs=1) as c_w, \
                     tc.tile_pool(name='c_t', bufs=3) as c_t, \
                     tc.tile_pool(name='c_ps', bufs=2, space='PSUM') as c_ps:
                    wo_sb = c_w.tile([128, HPC, C], bf16)
                    nc.sync.dma_start(
                        out=wo_sb[:],
                        in_=wout_d.ap().rearrange('(h p) c -> p h c', p=128))
                    for tt in range(NT):
                        cps = c_ps.tile([128, C], f32, tag='cps')
                        for h in range(HPC):
                            for lo, hi in ((0, 512), (512, 1024),
                                           (1024, C)):
                                nc.tensor.matmul(
                                    cps[:, lo:hi],
                                    attnT[:, h, tt * 128:(tt + 1) * 128],
                                    wo_sb[:, h, lo:hi],
                                    start=(h == 0), stop=(h == HPC - 1))
                        ob = c_t.tile([128, C], bf16, tag='ob')
                        nc.vector.tensor_copy(out=ob[:, 0:640],
                                              in_=cps[:, 0:640])
                        nc.scalar.copy(out=ob[:, 640:C], in_=cps[:, 640:C])
                        nc.sync.dma_start(
                            out=rs_in[tt * 128:(tt + 1) * 128, :], in_=ob[:])
                    nc.gpsimd.collective_compute(
                        'ReduceScatter', OP.add,
                        ins=[rs_in[:].opt()],
                        outs=[rs_out[:].opt()],
                        replica_groups=[[2 * i, 2 * i + 1]
                                        for i in range(n_cores // 2)])

            # ====== phase D: residual + MLP over my T/2 tokens ======
            with contextlib.ExitStack() as pd:
                d_t = pd.enter_context(tc.tile_pool(name='d_t', bufs=2))
                d_big = pd.enter_context(tc.tile_pool(name='d_big', bufs=1))
                x2_sb = d_big.tile([128, NH, C], f32)
                xn2T = d_big.tile([128, 10, T // 2], bf16)
                h2T = d_big.tile([128, F // 128, T // 2], bf16)

                with tc.tile_pool(name='d_ps', bufs=4, space='PSUM') as d_ps, \
                     tc.tile_pool(name='dt_ps', bufs=2,
                                  space='PSUM') as dt_ps:
                    for tt in range(NH):
                        rsx = d_t.tile([128, C], bf16, tag='rsx')
                        nc.gpsimd.dma_start(
                            out=rsx[:],
                            in_=rs_out[tt * 128:(tt + 1) * 128, :])
                        xht = d_t.tile([128, C], f32, tag='xht')
                        nc.sync.dma_start(
                            out=xht[:],
                            in_=xh_d[tt * 128:(tt + 1) * 128, :])
                        nc.vector.tensor_add(out=x2_sb[:, tt, :],
                                             in0=rsx[:], in1=xht[:])
                        ssq2 = d_t.tile([128, 1], f32, tag='ssq2')
                        nc.scalar.activation(out=scr_sq[:],
                                             in_=x2_sb[:, tt, :],
                                             func=AF.Square, bias=zero_sb[:],
                                             accum_out=ssq2[:])
                        rstd2 = d_t.tile([128, 1], f32, tag='rstd2')
                        nc.scalar.activation(out=rstd2[:], in_=ssq2[:],
                                             func=AF.Sqrt,
                                             scale=float(1.0 / C),
                                             bias=eps_sb[:])
                        nc.vector.reciprocal(out=rstd2[:], in_=rstd2[:])
                        xn2 = d_t.tile([128, C], bf16, tag='xn2')
                        nc.scalar.activation(out=xn2[:], in_=x2_sb[:, tt, :],
                                             func=AF.Copy, scale=rstd2[:])
                        for jg, (lo, hi) in enumerate(((0, 4), (4, 8),
                                                      (8, 10))):
                            tp2 = dt_ps.tile([128, 640], bf16, tag='tp2')
                            for j in range(lo, hi):
                                nc.tensor.transpose(
                                    tp2[:, (j - lo) * 128:(j - lo + 1) * 128],
                                    xn2[:, j * 128:(j + 1) * 128], ident_b[:])
                            nc.vector.tensor_copy(
                                out=xn2T[:, lo:hi, tt * 128:(tt + 1) * 128],
                                in_=tp2[:, 0:(hi - lo) * 128].rearrange(
                                    'p (j d) -> p j d', d=128))

                    # fc1: j-loop with LDW amortized over two 512 chunks
                    with tc.tile_pool(name='wf1', bufs=3) as wf1_pool:
                        for fi in range(F // 128):
                            wf1 = wf1_pool.tile([128, 10, 128], bf16,
                                                tag='wf1')
                            nc.sync.dma_start(
                                out=wf1[:],
                                in_=wfc1_d[:, fi * 128:(fi + 1) * 128]
                                .rearrange('(j p) c -> p j c', p=128))
                            hp0 = d_ps.tile([128, 512], f32, tag='hps')
                            hp1 = d_ps.tile([128, 512], f32, tag='hps')
                            for j in range(10):
                                nc.tensor.matmul(hp0[:], wf1[:, j, :],
                                                 xn2T[:, j, 0:512],
                                                 start=(j == 0),
                                                 stop=(j == 9))
                                nc.tensor.matmul(hp1[:], wf1[:, j, :],
                                                 xn2T[:, j, 512:1024],
                                                 start=(j == 0),
                                                 stop=(j == 9))
                            for ci, hp in ((0, hp0), (1, hp1)):
                                hrelu = d_t.tile([128, 512], bf16,
                                                 tag='hrelu')
                                nc.scalar.activation(out=hrelu[:], in_=hp[:],
                                                     func=AF.Relu,
                                                     bias=zero_sb[:])
                                nc.vector.tensor_mul(
                                    out=h2T[:, fi,
                                            ci * 512:(ci + 1) * 512],
                                    in0=hrelu[:], in1=hrelu[:])

                # fc2: tt-pairs, 3 matmuls (1280 cols) per lhsT
                with tc.tile_pool(name='y_ps', bufs=2, space='PSUM') as y_ps, \
                     tc.tile_pool(name='yt_ps', bufs=2,
                                  space='PSUM') as yt_ps, \
                     tc.tile_pool(name='wf2', bufs=3) as wf2_pool:
                    for ttg in range(NH // 2):
                        yps = [y_ps.tile([128, 1024], f32, tag='yps',
                                         name='yps')
                               for _ in range(2)]
                        ypt = [yt_ps.tile([128, 256], f32, tag='ypt',
                                          name='ypt')
                               for _ in range(2)]
                        for fi in range(F // 128):
                            wf2 = wf2_pool.tile([128, C], bf16, tag='wf2')
                            nc.sync.dma_start(
                                out=wf2[:],
                                in_=wfc2_d[fi * 128:(fi + 1) * 128, :])
                            for i in range(2):
                                tt = 2 * ttg + i
                                lhsT = h2T[:, fi, tt * 128:(tt + 1) * 128]
                                st = (fi == 0)
                                sp_ = (fi == F // 128 - 1)
                                nc.tensor.matmul(yps[i][:, 0:512], lhsT,
                                                 wf2[:, 0:512],
                                                 start=st, stop=sp_)
                                nc.tensor.matmul(yps[i][:, 512:1024], lhsT,
                                                 wf2[:, 512:1024],
                                                 start=st, stop=sp_)
                                nc.tensor.matmul(ypt[i][:], lhsT,
                                                 wf2[:, 1024:C],
                                                 start=st, stop=sp_)
                        for i in range(2):
                            tt = 2 * ttg + i
                            yo = d_t.tile([128, C], f32, tag='yo')
                            nc.vector.tensor_add(out=yo[:, 0:1024],
                                                 in0=yps[i][:],
                                                 in1=x2_sb[:, tt, 0:1024])
                            nc.vector.tensor_add(out=yo[:, 1024:C],
                                                 in0=ypt[i][:],
                                                 in1=x2_sb[:, tt, 1024:C])
                            nc.sync.dma_start(
                                out=y_d[tt * 128:(tt + 1) * 128, :],
                                in_=yo[:])

    nc.compile()
    return nc


_CACHE = {}


def _get_nc(t_len=T):
    if t_len not in _CACHE:
        _CACHE[t_len] = build_nc(t_len)
    return _CACHE[t_len]


def make_in_maps(x, rotary_pos_emb, ln1_w, w_qkv, qn_w, kn_w, w_out, ln2_w,
                 w_fc1, w_fc2, t_len=T):
    """Host-side sharding prep. Returns list of per-core input dicts."""
    x = np.asarray(x, np.float32)
    rot = np.asarray(rotary_pos_emb, np.float32)
    cos = np.cos(rot).astype(np.float32)
    sin = np.sin(rot).astype(np.float32)
    sinneg = np.concatenate([-sin[:, :64], sin[:, :64]], axis=-1)
    qn = np.asarray(qn_w, np.float32)
    kn = np.asarray(kn_w, np.float32)
    cosq = (cos * qn).astype(ml_dtypes.bfloat16)
    sinq = (sinneg * qn).astype(ml_dtypes.bfloat16)
    cosk = (cos * kn).astype(ml_dtypes.bfloat16)
    sink = (sinneg * kn).astype(ml_dtypes.bfloat16)
    w_qkv_f = (np.asarray(w_qkv, np.float32)
               * np.asarray(ln1_w, np.float32)[:, None]).reshape(C, 3, H, D)
    w_fc1_f = (np.asarray(w_fc1, np.float32)
               * np.asarray(ln2_w, np.float32)[:, None]
               ).astype(ml_dtypes.bfloat16)
    w_fc2_b = np.asarray(w_fc2, np.float32).astype(ml_dtypes.bfloat16)
    wo = np.asarray(w_out, np.float32).reshape(H, D, C)

    in_maps = []
    for c in range(N_CORES):
        b, hg = c // 2, c % 2
        heads = slice(hg * HPC, (hg + 1) * HPC)
        wq = np.ascontiguousarray(
            w_qkv_f[:, :, heads, :].reshape(C, 3 * CPC)
        ).astype(ml_dtypes.bfloat16)
        w_outp = np.ascontiguousarray(
            wo[heads].reshape(CPC, C)).astype(ml_dtypes.bfloat16)
        in_maps.append({
            'x': np.ascontiguousarray(x[b]),
            'xh': np.ascontiguousarray(x[b, hg * T // 2:(hg + 1) * T // 2]),
            'w_qkv': wq,
            'cosq': cosq, 'sinq': sinq, 'cosk': cosk, 'sink': sink,
            'w_out': w_outp,
            'w_fc1': np.ascontiguousarray(w_fc1_f),
            'w_fc2': np.ascontiguousarray(w_fc2_b),
        })
    return in_maps


def assemble_output(results, t_len=T):
    out = np.zeros((B, t_len, C), np.float32)
    for c in range(N_CORES):
        b, hg = c // 2, c % 2
        out[b, hg * t_len // 2:(hg + 1) * t_len // 2] = results[c]['y']
    return out


def kernel(**inputs):
    nc = _get_nc(T)
    in_maps = make_in_maps(**inputs)
    res = bass_utils.run_bass_kernel_spmd(nc, in_maps,
                                          core_ids=list(range(N_CORES)))
    return assemble_output(res.results)


# revision 11
# speedup vs baseline: 1.0834x; 1.0720x over previous
"""Trainium2 Bass kernel for nn_Block_27848567948000 (dense transformer block).

Sharding (8 NeuronCores): 4 data-parallel groups over batch (B=4), 2-way
tensor-parallel within each pair: attention sharded over heads (5 each).
out_proj computed as per-head partial sums over ALL T, summed + token-scattered
via a pairwise ReduceScatter; MLP over the core's T/2 token half.

kernel(**inputs) takes FULL inputs and returns the FULL (4, 2048, 1280) output.
"""
import sys

sys.path.insert(0, '/opt/trn_rl_repo')

import numpy as np
import ml_dtypes

import concourse.bass as bass
import concourse.tile as tile
from concourse import mybir, bacc
from concourse import bass_utils
from concourse.masks import make_identity

B, T, C, H, D, F = 4, 2048, 1280, 10, 128, 5120
EPS = 1e-5
N_CORES = 8
HPC = H // 2            # heads per core (5)
CPC = HPC * D           # channels per core (640)
f32 = mybir.dt.float32
bf16 = mybir.dt.bfloat16
fp8 = mybir.dt.float8e4
i32 = mybir.dt.int32
AF = mybir.ActivationFunctionType
OP = mybir.AluOpType
AX = mybir.AxisListType

NT = T // 128            # 16 token tiles
NH = T // 2 // 128       # 8 token tiles in my half
QB = T // 512            # 4 query blocks
NBLK = HPC * 4           # 20 mxfp8 blocks per tensor per token
INV_SQRT_D = float(1.0 / np.sqrt(D))
NEG = -30000.0


def _ap(t_ap, offset_delta, pattern):
    return bass.AP(tensor=t_ap.tensor, offset=t_ap.offset + offset_delta,
                   ap=pattern)


def build_nc(t_len=T, n_cores=N_CORES):
    import contextlib
    nc = bacc.Bacc('TRN2', target_bir_lowering=False, debug=False,
                   num_devices=n_cores)

    # ---- DRAM I/O ----
    x_d = nc.dram_tensor('x', [T, C], f32, kind='ExternalInput')
    xh_d = nc.dram_tensor('xh', [T // 2, C], f32, kind='ExternalInput')
    wqkv_d = nc.dram_tensor('w_qkv', [C, 3 * CPC], bf16, kind='ExternalInput')
    cosq_d = nc.dram_tensor('cosq', [T, D], bf16, kind='ExternalInput')
    sinq_d = nc.dram_tensor('sinq', [T, D], bf16, kind='ExternalInput')
    cosk_d = nc.dram_tensor('cosk', [T, D], bf16, kind='ExternalInput')
    sink_d = nc.dram_tensor('sink', [T, D], bf16, kind='ExternalInput')
    wout_d = nc.dram_tensor('w_out', [CPC, C], bf16, kind='ExternalInput')
    wfc1_d = nc.dram_tensor('w_fc1', [C, F], bf16, kind='ExternalInput')
    wfc2_d = nc.dram_tensor('w_fc2', [F, C], bf16, kind='ExternalInput')
    y_d = nc.dram_tensor('y', [T // 2, C], f32, kind='ExternalOutput')

    with tile.TileContext(nc) as tc:
        with contextlib.ExitStack() as ctx:
            persist = ctx.enter_context(tc.tile_pool(name='persist', bufs=1))
            dram = ctx.enter_context(tc.tile_pool(name='dram', bufs=1,
                                                  space='DRAM'))

            # ---- constants ----
            ident_b = persist.tile([128, 128], bf16)
            make_identity(nc, ident_b)
            ones128 = persist.tile([128, 128], bf16)
            nc.vector.memset(ones128[:], 1.0)
            zero_sb = persist.tile([128, 1], f32)
            nc.vector.memset(zero_sb[:], 0.0)
            eps_sb = persist.tile([128, 1], f32)
            nc.vector.memset(eps_sb[:], EPS)
            scr_sq = persist.tile([128, C], bf16)   # Square-output scratch

            # DRAM scratch for the collective
            rs_in = dram.tile([T, C], bf16)
            rs_out = dram.tile([T // 2, C], bf16)

            with contextlib.ExitStack() as pab:
                ab = pab.enter_context(tc.tile_pool(name='ab', bufs=1))
                qT = ab.tile([128, HPC, T], bf16)
                kT = ab.tile([128, HPC, T], bf16)
                vd_sb = ab.tile([128, NT, HPC, 130], bf16)
                attnT = ab.tile([128, HPC, T], bf16)
                nc.vector.memset(vd_sb[:, :, :, 128:129], 1.0)

                # ====== phases A+B ======
                with contextlib.ExitStack() as pin:
                    a_w = pin.enter_context(tc.tile_pool(name='a_w', bufs=1))
                    wq_sb = a_w.tile([128, 10, 3 * CPC], bf16)
                    nc.sync.dma_start(
                        out=wq_sb[:],
                        in_=wqkv_d.ap().rearrange('(j p) c -> p j c', p=128))
                    cq_sb = a_w.tile([128, NT, D], bf16)
                    nc.sync.dma_start(
                        out=cq_sb[:],
                        in_=cosq_d.ap().rearrange('(t p) d -> p t d', p=128))
                    sq_sb = a_w.tile([128, NT, D], bf16)
                    nc.sync.dma_start(
                        out=sq_sb[:],
                        in_=sinq_d.ap().rearrange('(t p) d -> p t d', p=128))
                    ck_sb = a_w.tile([128, NT, D], bf16)
                    nc.sync.dma_start(
                        out=ck_sb[:],
                        in_=cosk_d.ap().rearrange('(t p) d -> p t d', p=128))
                    sk_sb = a_w.tile([128, NT, D], bf16)
                    nc.sync.dma_start(
                        out=sk_sb[:],
                        in_=sink_d.ap().rearrange('(t p) d -> p t d', p=128))

                    a_t = pin.enter_context(tc.tile_pool(name='a_t', bufs=2))
                    a_s = pin.enter_context(tc.tile_pool(name='a_s', bufs=2))
                    a_q = pin.enter_context(tc.tile_pool(name='a_q', bufs=2))
                    pT_pool = pin.enter_context(
                        tc.tile_pool(name='pT', bufs=4))
                    b_t = pin.enter_context(tc.tile_pool(name='b_t', bufs=2))
                    ps512 = pin.enter_context(
                        tc.tile_pool(name='ps512', bufs=3, space='PSUM'))
                    ops_ps = pin.enter_context(
                        tc.tile_pool(name='ops_ps', bufs=1, space='PSUM'))
                    psT = pin.enter_context(
                        tc.tile_pool(name='psT', bufs=2, space='PSUM'))
                    psD = pin.enter_context(
                        tc.tile_pool(name='psD', bufs=2, space='PSUM'))

                    stash = {}

                    def emit_head(t):
                        xt = a_s.tile([128, C], f32, tag='xt')
                        nc.sync.dma_start(out=xt[:],
                                          in_=x_d[t * 128:(t + 1) * 128, :])
                        ssq = a_s.tile([128, 1], f32, tag='ssq')
                        nc.scalar.activation(out=scr_sq[:], in_=xt[:],
                                             func=AF.Square, bias=zero_sb[:],
                                             accum_out=ssq[:])
                        rstd = a_s.tile([128, 1], f32, tag='rstd')
                        nc.scalar.activation(out=rstd[:], in_=ssq[:],
                                             func=AF.Ln,
                                             scale=float(1.0 / C),
                                             bias=eps_sb[:])
                        nc.scalar.activation(out=rstd[:], in_=rstd[:],
                                             func=AF.Exp, scale=-0.5,
                                             bias=zero_sb[:])
                        xn = a_s.tile([128, C], bf16, tag='xn')
                        nc.scalar.activation(out=xn[:], in_=xt[:],
                                             func=AF.Copy, scale=rstd[:])
                        xnT = a_s.tile([128, 10, 128], bf16, tag='xnT')
                        for jg, (lo, hi) in enumerate(((0, 4), (4, 8),
                                                      (8, 10))):
                            tp = psT.tile([128, 640], bf16, tag='tp')
                            for j in range(lo, hi):
                                nc.tensor.transpose(
                                    tp[:, (j - lo) * 128:(j - lo + 1) * 128],
                                    xn[:, j * 128:(j + 1) * 128], ident_b[:])
                            nc.vector.tensor_copy(
                                out=xnT[:, lo:hi, :],
                                in_=tp[:, 0:(hi - lo) * 128].rearrange(
                                    'p (j d) -> p j d', d=128))
                        # QKV (chunk-outer, j-mid, g-inner: LDW amortized)
                        qf = a_q.tile([128, CPC], bf16, tag='qf')
                        kf = a_q.tile([128, CPC], bf16, tag='kf')
                        vf = a_q.tile([128, CPC], bf16, tag='vf')
                        dsts = (qf, kf, vf)
                        for lo, hi in ((0, 512), (512, 640)):
                            pss = [ps512.tile([128, 512], f32, tag='mm',
                                              name='qkvps')
                                   for _ in range(3)]
                            for j in range(10):
                                for g in range(3):
                                    nc.tensor.matmul(
                                        pss[g][:, 0:hi - lo], xnT[:, j, :],
                                        wq_sb[:, j,
                                              g * CPC + lo:g * CPC + hi],
                                        start=(j == 0), stop=(j == 9))
                            for g in range(3):
                                if g == 1:
                                    nc.scalar.copy(out=dsts[g][:, lo:hi],
                                                   in_=pss[g][:, 0:hi - lo])
                                else:
                                    nc.vector.tensor_copy(
                                        out=dsts[g][:, lo:hi],
                                        in_=pss[g][:, 0:hi - lo])
                        stash[t] = (qf, kf, vf)

                    def rope(eng, src, cos_t, sin_t, out):
                        # out[p,h,d] = src*cos + swap(src)*sinneg   (bf16)
                        src3 = src[:].rearrange('p (h d) -> p h d', h=HPC)
                        pa = list(src3.ap)
                        swap = _ap(src3, 64, pa[:2] + [[-64, 2], [1, 64]])
                        ca = list(cos_t.ap)
                        cos4 = _ap(cos_t, 0, [ca[0], [0, HPC], [1, 128]])
                        sin4 = _ap(sin_t, 0,
                                   [ca[0], [0, HPC], [64, 2], [1, 64]])
                        tmp = a_t.tile([128, HPC, D], bf16, tag='rtmp')
                        eng.tensor_tensor(
                            out=tmp[:].rearrange('p h (u d) -> p h u d', u=2),
                            in0=swap, in1=sin4, op=OP.mult)
                        eng.tensor_tensor(out=out[:], in0=src3, in1=cos4,
                                          op=OP.mult)
                        eng.tensor_add(out=out[:], in0=out[:], in1=tmp[:])

                    def blk4(ap20, reps=32):
                        # (128,20) -> (128,5,4,reps) block broadcast
                        a = list(ap20.ap)
                        st = a[-1][0]
                        return bass.AP(tensor=ap20.tensor, offset=ap20.offset,
                                       ap=[a[0], [4 * st, HPC], [st, 4],
                                           [0, reps]])

                    def hb(ap5, reps=4):
                        # (128,5) -> (128,5,reps) broadcast
                        a = list(ap5.ap)
                        return bass.AP(tensor=ap5.tensor, offset=ap5.offset,
                                       ap=[a[0], [a[-1][0], HPC], [0, reps]])

                    def v4(x):
                        return x.rearrange('p h (b e) -> p h b e', e=32)

                    def emit_tail(t):
                        qf, kf, vf = stash.pop(t)
                        # rms of pre-rope q/k (rope is norm-preserving)
                        msq = a_t.tile([128, 2, HPC], f32, tag='msq')
                        for i, src in enumerate((qf, kf)):
                            for h in range(HPC):
                                nc.scalar.activation(
                                    out=scr_sq[:, 0:D],
                                    in_=src[:, h * D:(h + 1) * D],
                                    func=AF.Square, bias=zero_sb[:],
                                    accum_out=msq[:, i, h:h + 1])
                        nc.scalar.activation(out=msq[:], in_=msq[:],
                                             func=AF.Ln,
                                             scale=float(1.0 / D),
                                             bias=eps_sb[:])
                        nc.scalar.activation(out=msq[:], in_=msq[:],
                                             func=AF.Exp, scale=-0.5,
                                             bias=zero_sb[:])
                        # rope (q on vector, k on gpsimd)
                        zq = a_t.tile([128, HPC, D], bf16, tag='zq')
                        rope(nc.vector, qf, cq_sb[:, t, :], sq_sb[:, t, :],
                             zq)
                        zk = a_t.tile([128, HPC, D], bf16, tag='zk')
                        rope(nc.gpsimd, kf, ck_sb[:, t, :], sk_sb[:, t, :],
                             zk)
                        # block amax; amn = amax*rstd (q,k) or amax (v)
                        amn = a_t.tile([128, 3, NBLK], f32, tag='amn')
                        nc.vector.tensor_reduce(
                            out=amn[:, 0, :], in_=v4(zq[:]), axis=AX.X,
                            op=OP.max, apply_absolute_value=True)
                        nc.vector.tensor_reduce(
                            out=amn[:, 1, :], in_=v4(zk[:]), axis=AX.X,
                            op=OP.max, apply_absolute_value=True)
                        nc.vector.tensor_reduce(
                            out=amn[:, 2, :],
                            in_=vf[:].rearrange('p (h b e) -> p h b e',
                                                h=HPC, e=32),
                            axis=AX.X, op=OP.max, apply_absolute_value=True)
                        for i in range(2):
                            nc.vector.tensor_tensor(
                                out=amn[:, i, :].rearrange(
                                    'p (h b) -> p h b', h=HPC),
                                in0=amn[:, i, :].rearrange(
                                    'p (h b) -> p h b', h=HPC),
                                in1=hb(msq[:, i, :]), op=OP.mult)
                        nc.vector.tensor_scalar_max(out=amn[:], in0=amn[:],
                                                    scalar1=1e-12)
                        eb = a_t.tile([128, 3, NBLK], i32, tag='eb')
                        nc.vector.tensor_single_scalar(
                            out=eb[:], in_=amn[:].bitcast(i32), scalar=23,
                            op=OP.logical_shift_right)
                        sc = a_t.tile([128, 3, NBLK], f32, tag='sc')
                        nc.vector.tensor_scalar(
                            out=sc[:].bitcast(i32), in0=eb[:], scalar1=-1,
                            scalar2=260, op0=OP.mult, op1=OP.add)
                        nc.vector.tensor_single_scalar(
                            out=sc[:].bitcast(i32), in_=sc[:].bitcast(i32),
                            scalar=23, op=OP.logical_shift_left)
                        isc = a_t.tile([128, 3, NBLK], f32, tag='isc')
                        nc.vector.tensor_single_scalar(
                            out=isc[:].bitcast(i32), in_=eb[:], scalar=6,
                            op=OP.subtract)
                        nc.vector.tensor_single_scalar(
                            out=isc[:].bitcast(i32), in_=isc[:].bitcast(i32),
                            scalar=23, op=OP.logical_shift_left)
                        msc = a_t.tile([128, 2, NBLK], f32, tag='msc')
                        for i in range(2):
                            nc.vector.tensor_tensor(
                                out=msc[:, i, :].rearrange(
                                    'p (h b) -> p h b', h=HPC),
                                in0=sc[:, i, :].rearrange(
                                    'p (h b) -> p h b', h=HPC),
                                in1=hb(msq[:, i, :]), op=OP.mult)
                        # quantize q (vector)
                        ys = a_t.tile([128, HPC, D], bf16, tag='ys')
                        q8 = a_t.tile([128, HPC, D], fp8, tag='q8')
                        qd = a_t.tile([128, HPC, D], bf16, tag='qd')
                        nc.vector.tensor_tensor(out=v4(ys[:]), in0=v4(zq[:]),
                                                in1=blk4(msc[:, 0, :]),
                                                op=OP.mult)
                        nc.vector.tensor_scalar(out=q8[:], in0=ys[:],
                                                scalar1=-112.0,
                                                scalar2=112.0,
                                                op0=OP.max, op1=OP.min)
                        nc.vector.tensor_tensor(out=v4(qd[:]), in0=v4(q8[:]),
                                                in1=blk4(isc[:, 0, :]),
                                                op=OP.mult)
                        # quantize k (gpsimd mults, vector fp8 cast)
                        ysk = a_t.tile([128, HPC, D], bf16, tag='ys')
                        k8 = a_t.tile([128, HPC, D], fp8, tag='q8')
                        kd = a_t.tile([128, HPC, D], bf16, tag='kd')
                        nc.gpsimd.tensor_tensor(out=v4(ysk[:]),
                                                in0=v4(zk[:]),
                                                in1=blk4(msc[:, 1, :]),
                                                op=OP.mult)
                        nc.vector.tensor_scalar(out=k8[:], in0=ysk[:],
                                                scalar1=-112.0,
                                                scalar2=112.0,
                                                op0=OP.max, op1=OP.min)
                        nc.gpsimd.tensor_tensor(out=v4(kd[:]), in0=v4(k8[:]),
                                                in1=blk4(isc[:, 1, :]),
                                                op=OP.mult)
                        # quantize v (vector; deq straight into vd_sb)
                        ysv = a_t.tile([128, HPC, D], bf16, tag='ys')
                        v8 = a_t.tile([128, HPC, D], fp8, tag='q8')
                        nc.vector.tensor_tensor(
                            out=v4(ysv[:]),
                            in0=v4(vf[:].rearrange('p (h d) -> p h d',
                                                   h=HPC)),
                            in1=blk4(sc[:, 2, :]), op=OP.mult)
                        nc.vector.tensor_scalar(out=v8[:], in0=ysv[:],
                                                scalar1=-112.0,
                                                scalar2=112.0,
                                                op0=OP.max, op1=OP.min)
                        nc.vector.tensor_tensor(
                            out=v4(vd_sb[:, t, :, 0:D]), in0=v4(v8[:]),
                            in1=blk4(isc[:, 2, :]), op=OP.mult)
                        # transpose qd/kd into qT/kT
                        for src, dstT in ((qd, qT), (kd, kT)):
                            tp = psT.tile([128, 640], bf16, tag='tp')
                            for h in range(HPC):
                                nc.tensor.transpose(
                                    tp[:, h * 128:(h + 1) * 128],
                                    src[:, h, :], ident_b[:])
                            nc.vector.tensor_copy(
                                out=dstT[:, :, t * 128:(t + 1) * 128],
                                in_=tp[:].rearrange('p (h d) -> p h d',
                                                    h=HPC))

                    def emit_attn(qb):
                        nkt = 4 * qb + 4
                        for h in range(HPC):
                            dps = psD.tile([128, 512], f32, tag='dps')
                            ops = ops_ps.tile([128, 512], f32, tag='ops')
                            for kt in range(nkt):
                                sp = ps512.tile([128, 512], f32, tag='mm')
                                o = kt - 4 * qb
                                nc.tensor.matmul(
                                    sp[:],
                                    kT[:, h, kt * 128:(kt + 1) * 128],
                                    qT[:, h, qb * 512:(qb + 1) * 512],
                                    start=True, stop=True)
                                pT = pT_pool.tile([128, 512], bf16, tag='pT')
                                nc.scalar.activation(out=pT[:], in_=sp[:],
                                                     func=AF.Exp,
                                                     bias=zero_sb[:],
                                                     scale=INV_SQRT_D)
                                if o >= 0:
                                    nc.gpsimd.affine_select(
                                        out=pT[:], in_=pT[:],
                                        compare_op=OP.is_ge, fill=0.0,
                                        base=-128 * o, pattern=[[1, 512]],
                                        channel_multiplier=-1)
                                nc.tensor.matmul(dps[:], ones128[:], pT[:],
                                                 start=(kt == 0),
                                                 stop=(kt == nkt - 1))
                                nc.tensor.matmul(ops[:],
                                                 vd_sb[:, kt, h, 0:128],
                                                 pT[:],
                                                 start=(kt == 0),
                                                 stop=(kt == nkt - 1))
                            rd = b_t.tile([128, 512], f32, tag='rd')
                            nc.vector.reciprocal_approx_fast(out=rd[:],
                                                             in_=dps[:])
                            nc.vector.tensor_tensor(
                                out=attnT[:, h, qb * 512:(qb + 1) * 512],
                                in0=ops[:], in1=rd[:], op=OP.mult)

                    # ---- interleaved A+B emission ----
                    for t in range(NT):
                        emit_head(t)
                        if t >= 1:
                            emit_tail(t - 1)
                        if t >= 4 and t % 4 == 0:
                            emit_attn(t // 4 - 1)
                    emit_tail(NT - 1)
                    emit_attn(QB - 1)

                # ====== phase C: out_proj partials + ReduceScatter ======
                with tc.tile_pool(name='c_w', bufs=1) as c_w, \
                     tc.tile_pool(name='c_t', bufs=3) as c_t, \
                     tc.tile_pool(name='c_ps', bufs=2, space='PSUM') as c_ps:
                    wo_sb = c_w.tile([128, HPC, C], bf16)
                    nc.sync.dma_start(
                        out=wo_sb[:],
                        in_=wout_d.ap().rearrange('(h p) c -> p h c', p=128))
                    # rs_in row layout: [t0:512 | t1024:1536 | t512:1024
                    # | t1536:2048] so each RS half is contiguous.
                    rowblk = {tt: i for i, tt in enumerate(
                        (0, 1, 2, 3, 8, 9, 10, 11, 4, 5, 6, 7,
                         12, 13, 14, 15))}

                    def oproj(tt):
                        cps = c_ps.tile([128, C], f32, tag='cps',
                                        name='cps')
                        for h in range(HPC):
                            for lo, hi in ((0, 512), (512, 1024),
                                           (1024, C)):
                                nc.tensor.matmul(
                                    cps[:, lo:hi],
                                    attnT[:, h, tt * 128:(tt + 1) * 128],
                                    wo_sb[:, h, lo:hi],
                                    start=(h == 0), stop=(h == HPC - 1))
                        ob = c_t.tile([128, C], bf16, tag='ob', name='ob')
                        nc.vector.tensor_copy(out=ob[:, 0:640],
                                              in_=cps[:, 0:640])
                        nc.scalar.copy(out=ob[:, 640:C], in_=cps[:, 640:C])
                        r = rowblk[tt]
                        nc.sync.dma_start(
                            out=rs_in[r * 128:(r + 1) * 128, :], in_=ob[:])

                    grp = [[2 * i, 2 * i + 1] for i in range(n_cores // 2)]
                    for tt in (0, 1, 2, 3, 8, 9, 10, 11):
                        oproj(tt)
                    nc.gpsimd.collective_compute(
                        'ReduceScatter', OP.add,
                        ins=[rs_in[0:1024, :].opt()],
                        outs=[rs_out[0:512, :].opt()], replica_groups=grp)
                    for tt in (4, 5, 6, 7, 12, 13, 14, 15):
                        oproj(tt)
                    nc.gpsimd.collective_compute(
                        'ReduceScatter', OP.add,
                        ins=[rs_in[1024:2048, :].opt()],
                        outs=[rs_out[512:1024, :].opt()],
                        replica_groups=grp)

            # ====== phase D: residual + MLP over my T/2 tokens ======
            with contextlib.ExitStack() as pd:
                d_t = pd.enter_context(tc.tile_pool(name='d_t', bufs=2))
                d_big = pd.enter_context(tc.tile_pool(name='d_big', bufs=1))
                x2_sb = d_big.tile([128, NH, C], f32)
                xn2T = d_big.tile([128, 10, T // 2], bf16)
                h2T = d_big.tile([128, F // 128, T // 2], bf16)

                with tc.tile_pool(name='d_ps', bufs=4, space='PSUM') as d_ps, \
                     tc.tile_pool(name='dt_ps', bufs=2,
                                  space='PSUM') as dt_ps:
                    for tt in range(NH):
                        rsx = d_t.tile([128, C], bf16, tag='rsx')
                        nc.gpsimd.dma_start(
                            out=rsx[:],
                            in_=rs_out[tt * 128:(tt + 1) * 128, :])
                        xht = d_t.tile([128, C], f32, tag='xht')
                        nc.sync.dma_start(
                            out=xht[:],
                            in_=xh_d[tt * 128:(tt + 1) * 128, :])
                        nc.vector.tensor_add(out=x2_sb[:, tt, :],
                                             in0=rsx[:], in1=xht[:])
                        ssq2 = d_t.tile([128, 1], f32, tag='ssq2')
                        nc.scalar.activation(out=scr_sq[:],
                                             in_=x2_sb[:, tt, :],
                                             func=AF.Square, bias=zero_sb[:],
                                             accum_out=ssq2[:])
                        rstd2 = d_t.tile([128, 1], f32, tag='rstd2')
                        nc.scalar.activation(out=rstd2[:], in_=ssq2[:],
                                             func=AF.Ln,
                                             scale=float(1.0 / C),
                                             bias=eps_sb[:])
                        nc.scalar.activation(out=rstd2[:], in_=rstd2[:],
                                             func=AF.Exp, scale=-0.5,
                                             bias=zero_sb[:])
                        xn2 = d_t.tile([128, C], bf16, tag='xn2')
                        nc.scalar.activation(out=xn2[:], in_=x2_sb[:, tt, :],
                                             func=AF.Copy, scale=rstd2[:])
                        for jg, (lo, hi) in enumerate(((0, 4), (4, 8),
                                                      (8, 10))):
                            tp2 = dt_ps.tile([128, 640], bf16, tag='tp2')
                            for j in range(lo, hi):
                                nc.tensor.transpose(
                                    tp2[:, (j - lo) * 128:(j - lo + 1) * 128],
                                    xn2[:, j * 128:(j + 1) * 128], ident_b[:])
                            nc.vector.tensor_copy(
                                out=xn2T[:, lo:hi, tt * 128:(tt + 1) * 128],
                                in_=tp2[:, 0:(hi - lo) * 128].rearrange(
                                    'p (j d) -> p j d', d=128))

                    # fc1: j-loop with LDW amortized over two 512 chunks
                    with tc.tile_pool(name='wf1', bufs=3) as wf1_pool:
                        for fi in range(F // 128):
                            wf1 = wf1_pool.tile([128, 10, 128], bf16,
                                                tag='wf1')
                            nc.sync.dma_start(
                                out=wf1[:],
                                in_=wfc1_d[:, fi * 128:(fi + 1) * 128]
                                .rearrange('(j p) c -> p j c', p=128))
                            hp0 = d_ps.tile([128, 512], f32, tag='hps')
                            hp1 = d_ps.tile([128, 512], f32, tag='hps')
                            for j in range(10):
                                nc.tensor.matmul(hp0[:], wf1[:, j, :],
                                                 xn2T[:, j, 0:512],
                                                 start=(j == 0),
                                                 stop=(j == 9))
                                nc.tensor.matmul(hp1[:], wf1[:, j, :],
                                                 xn2T[:, j, 512:1024],
                                                 start=(j == 0),
                                                 stop=(j == 9))
                            for ci, hp in ((0, hp0), (1, hp1)):
                                hrelu = d_t.tile([128, 512], bf16,
                                                 tag='hrelu')
                                nc.scalar.activation(out=hrelu[:], in_=hp[:],
                                                     func=AF.Relu,
                                                     bias=zero_sb[:])
                                nc.vector.tensor_mul(
                                    out=h2T[:, fi,
                                            ci * 512:(ci + 1) * 512],
                                    in0=hrelu[:], in1=hrelu[:])

                # fc2: c-halves x tt-quads; 2 matmuls (640 cols) per lhsT
                with tc.tile_pool(name='y_ps', bufs=4, space='PSUM') as y_ps, \
                     tc.tile_pool(name='wf2', bufs=3) as wf2_pool:
                    for clo, chi in ((0, 640), (640, C)):
                        for ttg in range(2):
                            yps = [y_ps.tile([128, 640], f32, tag='yps',
                                             name='yps')
                                   for _ in range(4)]
                            for fi in range(F // 128):
                                wf2 = wf2_pool.tile([128, 640], bf16,
                                                    tag='wf2')
                                nc.sync.dma_start(
                                    out=wf2[:],
                                    in_=wfc2_d[fi * 128:(fi + 1) * 128,
                                               clo:chi])
                                for i in range(4):
                                    tt = 4 * ttg + i
                                    lhsT = h2T[:, fi,
                                               tt * 128:(tt + 1) * 128]
                                    st = (fi == 0)
                                    sp_ = (fi == F // 128 - 1)
                                    nc.tensor.matmul(yps[i][:, 0:512], lhsT,
                                                     wf2[:, 0:512],
                                                     start=st, stop=sp_)
                                    nc.tensor.matmul(yps[i][:, 512:640],
                                                     lhsT,
                                                     wf2[:, 512:640],
                                                     start=st, stop=sp_)
                            for i in range(4):
                                tt = 4 * ttg + i
                                yo = d_t.tile([128, 640], f32, tag='yo')
                                nc.vector.tensor_add(
                                    out=yo[:], in0=yps[i][:],
                                    in1=x2_sb[:, tt, clo:chi])
                                nc.sync.dma_start(
                                    out=y_d[tt * 128:(tt + 1) * 128,
                                            clo:chi],
                                    in_=yo[:])

    nc.compile()
    return nc


_CACHE = {}


def _get_nc(t_len=T):
    if t_len not in _CACHE:
        _CACHE[t_len] = build_nc(t_len)
    return _CACHE[t_len]


def make_in_maps(x, rotary_pos_emb, ln1_w, w_qkv, qn_w, kn_w, w_out, ln2_w,
                 w_fc1, w_fc2, t_len=T):
    """Host-side sharding prep. Returns list of per-core input dicts."""
    x = np.asarray(x, np.float32)
    rot = np.asarray(rotary_pos_emb, np.float32)
    cos = np.cos(rot).astype(np.float32)
    sin = np.sin(rot).astype(np.float32)
    sinneg = np.concatenate([-sin[:, :64], sin[:, :64]], axis=-1)
    qn = np.asarray(qn_w, np.float32)
    kn = np.asarray(kn_w, np.float32)
    cosq = (cos * qn).astype(ml_dtypes.bfloat16)
    sinq = (sinneg * qn).astype(ml_dtypes.bfloat16)
    cosk = (cos * kn).astype(ml_dtypes.bfloat16)
    sink = (sinneg * kn).astype(ml_dtypes.bfloat16)
    w_qkv_f = (np.asarray(w_qkv, np.float32)
               * np.asarray(ln1_w, np.float32)[:, None]).reshape(C, 3, H, D)
    w_fc1_f = (np.asarray(w_fc1, np.float32)
               * np.asarray(ln2_w, np.float32)[:, None]
               ).astype(ml_dtypes.bfloat16)
    w_fc2_b = np.asarray(w_fc2, np.float32).astype(ml_dtypes.bfloat16)
    wo = np.asarray(w_out, np.float32).reshape(H, D, C)

    in_maps = []
    for c in range(N_CORES):
        b, hg = c // 2, c % 2
        heads = slice(hg * HPC, (hg + 1) * HPC)
        wq = np.ascontiguousarray(
            w_qkv_f[:, :, heads, :].reshape(C, 3 * CPC)
        ).astype(ml_dtypes.bfloat16)
        w_outp = np.ascontiguousarray(
            wo[heads].reshape(CPC, C)).astype(ml_dtypes.bfloat16)
        in_maps.append({
            'x': np.ascontiguousarray(x[b]),
            'xh': np.ascontiguousarray(x[b, hg * T // 2:(hg + 1) * T // 2]),
            'w_qkv': wq,
            'cosq': cosq, 'sinq': sinq, 'cosk': cosk, 'sink': sink,
            'w_out': w_outp,
            'w_fc1': np.ascontiguousarray(w_fc1_f),
            'w_fc2': np.ascontiguousarray(w_fc2_b),
        })
    return in_maps


def assemble_output(results, t_len=T):
    out = np.zeros((B, t_len, C), np.float32)
    for c in range(N_CORES):
        b, hg = c // 2, c % 2
        out[b, hg * t_len // 2:(hg + 1) * t_len // 2] = results[c]['y']
    return out


def kernel(**inputs):
    nc = _get_nc(T)
    in_maps = make_in_maps(**inputs)
    res = bass_utils.run_bass_kernel_spmd(nc, in_maps,
                                          core_ids=list(range(N_CORES)))
    return assemble_output(res.results)


# revision 13
# speedup vs baseline: 1.2839x; 1.1851x over previous
"""Trainium2 Bass kernel for nn_Block_27848567948000 (dense transformer block).

Sharding (8 NeuronCores): 4 data-parallel groups over batch (B=4), 2-way
tensor-parallel within each pair: attention sharded over heads (5 each).
out_proj computed as per-head partial sums over ALL T, summed + token-scattered
via a pairwise ReduceScatter; MLP over the core's T/2 token half.

kernel(**inputs) takes FULL inputs and returns the FULL (4, 2048, 1280) output.
"""
import sys

sys.path.insert(0, '/opt/trn_rl_repo')

import numpy as np
import ml_dtypes

import concourse.bass as bass
import concourse.tile as tile
from concourse import mybir, bacc
from concourse import bass_utils
from concourse.masks import make_identity

B, T, C, H, D, F = 4, 2048, 1280, 10, 128, 5120
EPS = 1e-5
N_CORES = 8
HPC = H // 2            # heads per core (5)
CPC = HPC * D           # channels per core (640)
f32 = mybir.dt.float32
bf16 = mybir.dt.bfloat16
fp8 = mybir.dt.float8e4
i32 = mybir.dt.int32
AF = mybir.ActivationFunctionType
OP = mybir.AluOpType
AX = mybir.AxisListType

NT = T // 128            # 16 token tiles
NH = T // 2 // 128       # 8 token tiles in my half
QB = T // 512            # 4 query blocks
NBLK = HPC * 4           # 20 mxfp8 blocks per tensor per token
INV_SQRT_D = float(1.0 / np.sqrt(D))
NEG = -30000.0


def _ap(t_ap, offset_delta, pattern):
    return bass.AP(tensor=t_ap.tensor, offset=t_ap.offset + offset_delta,
                   ap=pattern)


def build_nc(t_len=T, n_cores=N_CORES):
    import contextlib
    nc = bacc.Bacc('TRN2', target_bir_lowering=False, debug=False,
                   num_devices=n_cores)

    # ---- DRAM I/O ----
    x_d = nc.dram_tensor('x', [T, C], f32, kind='ExternalInput')
    xh_d = nc.dram_tensor('xh', [T // 2, C], f32, kind='ExternalInput')
    wqkv_d = nc.dram_tensor('w_qkv', [C, 3 * CPC], bf16, kind='ExternalInput')
    cosq_d = nc.dram_tensor('cosq', [T, D], bf16, kind='ExternalInput')
    sinq_d = nc.dram_tensor('sinq', [T, D], bf16, kind='ExternalInput')
    cosk_d = nc.dram_tensor('cosk', [T, D], bf16, kind='ExternalInput')
    sink_d = nc.dram_tensor('sink', [T, D], bf16, kind='ExternalInput')
    wout_d = nc.dram_tensor('w_out', [CPC, C], bf16, kind='ExternalInput')
    wfc1_d = nc.dram_tensor('w_fc1', [C, F], bf16, kind='ExternalInput')
    wfc2_d = nc.dram_tensor('w_fc2', [F, C], bf16, kind='ExternalInput')
    y_d = nc.dram_tensor('y', [T // 2, C], f32, kind='ExternalOutput')

    with tile.TileContext(nc) as tc:
        with contextlib.ExitStack() as ctx:
            persist = ctx.enter_context(tc.tile_pool(name='persist', bufs=1))
            dram = ctx.enter_context(tc.tile_pool(name='dram', bufs=1,
                                                  space='DRAM'))

            # ---- constants ----
            ident_b = persist.tile([128, 128], bf16)
            make_identity(nc, ident_b)
            ident_f = persist.tile([128, 128], f32)
            make_identity(nc, ident_f)
            ones128 = persist.tile([128, 128], bf16)
            nc.vector.memset(ones128[:], 1.0)
            zero_sb = persist.tile([128, 1], f32)
            nc.vector.memset(zero_sb[:], 0.0)
            eps_sb = persist.tile([128, 1], f32)
            nc.vector.memset(eps_sb[:], EPS)
            scr_sq = persist.tile([128, C], bf16)   # Square-output scratch

            # DRAM scratch for the collective
            rs_in = dram.tile([T, C], bf16)
            rs_out = dram.tile([T // 2, C], bf16)

            with contextlib.ExitStack() as pab:
                ab = pab.enter_context(tc.tile_pool(name='ab', bufs=1))
                qT = ab.tile([128, HPC, T], bf16)
                kT = ab.tile([128, HPC, T], bf16)
                vd_sb = ab.tile([128, NT, HPC, 130], bf16)
                attnT = ab.tile([128, HPC, T], bf16)
                nc.vector.memset(vd_sb[:, :, :, 128:129], 1.0)

                # ====== phases A+B ======
                with contextlib.ExitStack() as pin:
                    a_w = pin.enter_context(tc.tile_pool(name='a_w', bufs=1))
                    wq_sb = a_w.tile([128, 10, 3 * CPC], bf16)
                    nc.sync.dma_start(
                        out=wq_sb[:],
                        in_=wqkv_d.ap().rearrange('(j p) c -> p j c', p=128))
                    cq_sb = a_w.tile([128, NT, D], bf16)
                    nc.sync.dma_start(
                        out=cq_sb[:],
                        in_=cosq_d.ap().rearrange('(t p) d -> p t d', p=128))
                    sq_sb = a_w.tile([128, NT, D], bf16)
                    nc.sync.dma_start(
                        out=sq_sb[:],
                        in_=sinq_d.ap().rearrange('(t p) d -> p t d', p=128))
                    ck_sb = a_w.tile([128, NT, D], bf16)
                    nc.sync.dma_start(
                        out=ck_sb[:],
                        in_=cosk_d.ap().rearrange('(t p) d -> p t d', p=128))
                    sk_sb = a_w.tile([128, NT, D], bf16)
                    nc.sync.dma_start(
                        out=sk_sb[:],
                        in_=sink_d.ap().rearrange('(t p) d -> p t d', p=128))

                    a_t = pin.enter_context(tc.tile_pool(name='a_t', bufs=2))
                    a_s = pin.enter_context(tc.tile_pool(name='a_s', bufs=2))
                    a_q = pin.enter_context(tc.tile_pool(name='a_q', bufs=2))
                    pT_pool = pin.enter_context(
                        tc.tile_pool(name='pT', bufs=3))
                    b_t = pin.enter_context(tc.tile_pool(name='b_t', bufs=2))
                    ps512 = pin.enter_context(
                        tc.tile_pool(name='ps512', bufs=3, space='PSUM'))
                    ops_ps = pin.enter_context(
                        tc.tile_pool(name='ops_ps', bufs=1, space='PSUM'))
                    psT = pin.enter_context(
                        tc.tile_pool(name='psT', bufs=3, space='PSUM'))
                    psD = pin.enter_context(
                        tc.tile_pool(name='psD', bufs=1, space='PSUM'))

                    stash = {}

                    def emit_head(t):
                        xt = a_s.tile([128, C], f32, tag='xt')
                        nc.sync.dma_start(out=xt[:],
                                          in_=x_d[t * 128:(t + 1) * 128, :])
                        ssq = a_s.tile([128, 1], f32, tag='ssq')
                        nc.scalar.activation(out=scr_sq[:], in_=xt[:],
                                             func=AF.Square, bias=zero_sb[:],
                                             accum_out=ssq[:])
                        rstd = a_s.tile([128, 1], f32, tag='rstd')
                        nc.scalar.activation(out=rstd[:], in_=ssq[:],
                                             func=AF.Ln,
                                             scale=float(1.0 / C),
                                             bias=eps_sb[:])
                        nc.scalar.activation(out=rstd[:], in_=rstd[:],
                                             func=AF.Exp, scale=-0.5,
                                             bias=zero_sb[:])
                        xnT = a_s.tile([128, 10, 128], bf16, tag='xnT')
                        for jg, (lo, hi) in enumerate(((0, 4), (4, 8),
                                                      (8, 10))):
                            tp = psT.tile([128, 512], f32, tag='tp',
                                          name='tpf')
                            for j in range(lo, hi):
                                nc.tensor.transpose(
                                    tp[:, (j - lo) * 128:(j - lo + 1) * 128],
                                    xt[:, j * 128:(j + 1) * 128], ident_f[:])
                            nc.vector.tensor_copy(
                                out=xnT[:, lo:hi, :],
                                in_=tp[:, 0:(hi - lo) * 128].rearrange(
                                    'p (j d) -> p j d', d=128))
                        # QKV (chunk-outer, j-mid, g-inner: LDW amortized)
                        qf = a_q.tile([128, CPC], bf16, tag='qf')
                        kf = a_q.tile([128, CPC], bf16, tag='kf')
                        vf = a_q.tile([128, CPC], bf16, tag='vf')
                        dsts = (qf, kf, vf)
                        for lo, hi in ((0, 512), (512, 640)):
                            pss = [ps512.tile([128, 512], f32, tag='mm',
                                              name='qkvps')
                                   for _ in range(3)]
                            for j in range(10):
                                for g in range(3):
                                    nc.tensor.matmul(
                                        pss[g][:, 0:hi - lo], xnT[:, j, :],
                                        wq_sb[:, j,
                                              g * CPC + lo:g * CPC + hi],
                                        start=(j == 0), stop=(j == 9))
                            for g in range(3):
                                if g == 1:
                                    nc.scalar.activation(
                                        out=dsts[g][:, lo:hi],
                                        in_=pss[g][:, 0:hi - lo],
                                        func=AF.Copy, scale=rstd[:])
                                else:
                                    nc.vector.tensor_scalar_mul(
                                        out=dsts[g][:, lo:hi],
                                        in0=pss[g][:, 0:hi - lo],
                                        scalar1=rstd[:])
                        stash[t] = (qf, kf, vf)

                    def rope(eng, src, cos_t, sin_t, out):
                        # out[p,h,d] = src*cos + swap(src)*sinneg   (bf16)
                        src3 = src[:].rearrange('p (h d) -> p h d', h=HPC)
                        pa = list(src3.ap)
                        swap = _ap(src3, 64, pa[:2] + [[-64, 2], [1, 64]])
                        ca = list(cos_t.ap)
                        cos4 = _ap(cos_t, 0, [ca[0], [0, HPC], [1, 128]])
                        sin4 = _ap(sin_t, 0,
                                   [ca[0], [0, HPC], [64, 2], [1, 64]])
                        tmp = a_t.tile([128, HPC, D], bf16, tag='rtmp')
                        eng.tensor_tensor(
                            out=tmp[:].rearrange('p h (u d) -> p h u d', u=2),
                            in0=swap, in1=sin4, op=OP.mult)
                        eng.tensor_tensor(out=out[:], in0=src3, in1=cos4,
                                          op=OP.mult)
                        eng.tensor_add(out=out[:], in0=out[:], in1=tmp[:])

                    def blk4(ap20, reps=32):
                        # (128,20) -> (128,5,4,reps) block broadcast
                        a = list(ap20.ap)
                        st = a[-1][0]
                        return bass.AP(tensor=ap20.tensor, offset=ap20.offset,
                                       ap=[a[0], [4 * st, HPC], [st, 4],
                                           [0, reps]])

                    def hb(ap5, reps=4):
                        # (128,5) -> (128,5,reps) broadcast
                        a = list(ap5.ap)
                        return bass.AP(tensor=ap5.tensor, offset=ap5.offset,
                                       ap=[a[0], [a[-1][0], HPC], [0, reps]])

                    def v4(x):
                        return x.rearrange('p h (b e) -> p h b e', e=32)

                    def emit_tail(t):
                        qf, kf, vf = stash.pop(t)
                        # rms of pre-rope q/k (rope is norm-preserving)
                        msq = a_t.tile([128, 2, HPC], f32, tag='msq')
                        for h in range(HPC):
                            nc.scalar.activation(
                                out=scr_sq[:, 0:D],
                                in_=qf[:, h * D:(h + 1) * D],
                                func=AF.Square, bias=zero_sb[:],
                                accum_out=msq[:, 0, h:h + 1])
                        ksq = a_t.tile([128, HPC, D], bf16, tag='ksq')
                        kf3 = kf[:].rearrange('p (h d) -> p h d', h=HPC)
                        nc.vector.tensor_tensor(out=ksq[:], in0=kf3,
                                                in1=kf3, op=OP.mult)
                        nc.vector.tensor_reduce(out=msq[:, 1, :],
                                                in_=ksq[:], axis=AX.X,
                                                op=OP.add)
                        nc.scalar.activation(out=msq[:], in_=msq[:],
                                             func=AF.Ln,
                                             scale=float(1.0 / D),
                                             bias=eps_sb[:])
                        nc.scalar.activation(out=msq[:], in_=msq[:],
                                             func=AF.Exp, scale=-0.5,
                                             bias=zero_sb[:])
                        # rope (q on vector, k on gpsimd)
                        zq = a_t.tile([128, HPC, D], bf16, tag='zq')
                        rope(nc.vector, qf, cq_sb[:, t, :], sq_sb[:, t, :],
                             zq)
                        zk = a_t.tile([128, HPC, D], bf16, tag='zk')
                        rope(nc.gpsimd, kf, ck_sb[:, t, :], sk_sb[:, t, :],
                             zk)
                        # block amax; amn = amax*rstd (q,k) or amax (v)
                        amn = a_t.tile([128, 3, NBLK], f32, tag='amn')
                        nc.vector.tensor_reduce(
                            out=amn[:, 0, :], in_=v4(zq[:]), axis=AX.X,
                            op=OP.max, apply_absolute_value=True)
                        nc.vector.tensor_reduce(
                            out=amn[:, 1, :], in_=v4(zk[:]), axis=AX.X,
                            op=OP.max, apply_absolute_value=True)
                        nc.vector.tensor_reduce(
                            out=amn[:, 2, :],
                            in_=vf[:].rearrange('p (h b e) -> p h b e',
                                                h=HPC, e=32),
                            axis=AX.X, op=OP.max, apply_absolute_value=True)
                        for i in range(2):
                            nc.vector.tensor_tensor(
                                out=amn[:, i, :].rearrange(
                                    'p (h b) -> p h b', h=HPC),
                                in0=amn[:, i, :].rearrange(
                                    'p (h b) -> p h b', h=HPC),
                                in1=hb(msq[:, i, :]), op=OP.mult)
                        nc.vector.tensor_scalar_max(out=amn[:], in0=amn[:],
                                                    scalar1=1e-12)
                        eb = a_t.tile([128, 3, NBLK], i32, tag='eb')
                        nc.vector.tensor_single_scalar(
                            out=eb[:], in_=amn[:].bitcast(i32), scalar=23,
                            op=OP.logical_shift_right)
                        sc = a_t.tile([128, 3, NBLK], f32, tag='sc')
                        nc.vector.tensor_scalar(
                            out=sc[:].bitcast(i32), in0=eb[:], scalar1=-1,
                            scalar2=260, op0=OP.mult, op1=OP.add)
                        nc.vector.tensor_single_scalar(
                            out=sc[:].bitcast(i32), in_=sc[:].bitcast(i32),
                            scalar=23, op=OP.logical_shift_left)
                        isc = a_t.tile([128, 3, NBLK], f32, tag='isc')
                        nc.vector.tensor_single_scalar(
                            out=isc[:].bitcast(i32), in_=eb[:], scalar=6,
                            op=OP.subtract)
                        nc.vector.tensor_single_scalar(
                            out=isc[:].bitcast(i32), in_=isc[:].bitcast(i32),
                            scalar=23, op=OP.logical_shift_left)
                        msc = a_t.tile([128, 2, NBLK], f32, tag='msc')
                        for i in range(2):
                            nc.vector.tensor_tensor(
                                out=msc[:, i, :].rearrange(
                                    'p (h b) -> p h b', h=HPC),
                                in0=sc[:, i, :].rearrange(
                                    'p (h b) -> p h b', h=HPC),
                                in1=hb(msq[:, i, :]), op=OP.mult)
                        # quantize q (vector)
                        ys = a_t.tile([128, HPC, D], bf16, tag='ys')
                        q8 = a_t.tile([128, HPC, D], fp8, tag='q8')
                        qd = a_t.tile([128, HPC, D], bf16, tag='qd')
                        nc.vector.tensor_tensor(out=v4(ys[:]), in0=v4(zq[:]),
                                                in1=blk4(msc[:, 0, :]),
                                                op=OP.mult)
                        nc.vector.tensor_scalar(out=q8[:], in0=ys[:],
                                                scalar1=-112.0,
                                                scalar2=112.0,
                                                op0=OP.max, op1=OP.min)
                        nc.vector.tensor_tensor(out=v4(qd[:]), in0=v4(q8[:]),
                                                in1=blk4(isc[:, 0, :]),
                                                op=OP.mult)
                        # quantize k (gpsimd mults, vector fp8 cast)
                        ysk = a_t.tile([128, HPC, D], bf16, tag='ys')
                        k8 = a_t.tile([128, HPC, D], fp8, tag='q8')
                        kd = a_t.tile([128, HPC, D], bf16, tag='kd')
                        nc.gpsimd.tensor_tensor(out=v4(ysk[:]),
                                                in0=v4(zk[:]),
                                                in1=blk4(msc[:, 1, :]),
                                                op=OP.mult)
                        nc.vector.tensor_scalar(out=k8[:], in0=ysk[:],
                                                scalar1=-112.0,
                                                scalar2=112.0,
                                                op0=OP.max, op1=OP.min)
                        nc.gpsimd.tensor_tensor(out=v4(kd[:]), in0=v4(k8[:]),
                                                in1=blk4(isc[:, 1, :]),
                                                op=OP.mult)
                        # quantize v (vector; deq straight into vd_sb)
                        ysv = a_t.tile([128, HPC, D], bf16, tag='ys')
                        v8 = a_t.tile([128, HPC, D], fp8, tag='q8')
                        nc.vector.tensor_tensor(
                            out=v4(ysv[:]),
                            in0=v4(vf[:].rearrange('p (h d) -> p h d',
                                                   h=HPC)),
                            in1=blk4(sc[:, 2, :]), op=OP.mult)
                        nc.vector.tensor_scalar(out=v8[:], in0=ysv[:],
                                                scalar1=-112.0,
                                                scalar2=112.0,
                                                op0=OP.max, op1=OP.min)
                        nc.vector.tensor_tensor(
                            out=v4(vd_sb[:, t, :, 0:D]), in0=v4(v8[:]),
                            in1=blk4(isc[:, 2, :]), op=OP.mult)
                        # transpose qd/kd into qT/kT
                        for src, dstT in ((qd, qT), (kd, kT)):
                            tp = psT.tile([128, 640], bf16, tag='tp')
                            for h in range(HPC):
                                nc.tensor.transpose(
                                    tp[:, h * 128:(h + 1) * 128],
                                    src[:, h, :], ident_b[:])
                            nc.vector.tensor_copy(
                                out=dstT[:, :, t * 128:(t + 1) * 128],
                                in_=tp[:].rearrange('p (h d) -> p h d',
                                                    h=HPC))

                    def emit_attn(qb):
                        nkt = 4 * qb + 4
                        for h in range(HPC):
                            dps = psD.tile([128, 512], f32, tag='dps')
                            ops = ops_ps.tile([128, 512], f32, tag='ops')
                            for kt in range(nkt):
                                sp = ps512.tile([128, 512], f32, tag='mm')
                                o = kt - 4 * qb
                                nc.tensor.matmul(
                                    sp[:],
                                    kT[:, h, kt * 128:(kt + 1) * 128],
                                    qT[:, h, qb * 512:(qb + 1) * 512],
                                    start=True, stop=True)
                                pT = pT_pool.tile([128, 512], bf16, tag='pT')
                                nc.scalar.activation(out=pT[:], in_=sp[:],
                                                     func=AF.Exp,
                                                     bias=zero_sb[:],
                                                     scale=INV_SQRT_D)
                                if o >= 0:
                                    nc.gpsimd.affine_select(
                                        out=pT[:], in_=pT[:],
                                        compare_op=OP.is_ge, fill=0.0,
                                        base=-128 * o, pattern=[[1, 512]],
                                        channel_multiplier=-1)
                                nc.tensor.matmul(dps[:], ones128[:], pT[:],
                                                 start=(kt == 0),
                                                 stop=(kt == nkt - 1))
                                nc.tensor.matmul(ops[:],
                                                 vd_sb[:, kt, h, 0:128],
                                                 pT[:],
                                                 start=(kt == 0),
                                                 stop=(kt == nkt - 1))
                            rd = b_t.tile([128, 512], f32, tag='rd')
                            nc.vector.reciprocal_approx_fast(out=rd[:],
                                                             in_=dps[:])
                            nc.vector.tensor_tensor(
                                out=attnT[:, h, qb * 512:(qb + 1) * 512],
                                in0=ops[:], in1=rd[:], op=OP.mult)

                    wo_sb = a_w.tile([128, HPC, C], bf16)
                    nc.sync.dma_start(
                        out=wo_sb[:],
                        in_=wout_d.ap().rearrange('(h p) c -> p h c', p=128))
                    # rs_in row layout: [t0:512 | t1024:1536 | t512:1024
                    # | t1536:2048] so each RS half is contiguous.
                    rowblk = {tt: i for i, tt in enumerate(
                        (0, 1, 2, 3, 8, 9, 10, 11, 4, 5, 6, 7,
                         12, 13, 14, 15))}
                    grp = [[2 * i, 2 * i + 1] for i in range(n_cores // 2)]

                    def oproj(tt):
                        ob = b_t.tile([128, C], bf16, tag='ob', name='ob')
                        for ci, (lo, hi) in enumerate(((0, 512),
                                                       (512, 1024),
                                                       (1024, C))):
                            ps = ps512.tile([128, 512], f32, tag='mm',
                                            name='oprojps')
                            for h in range(HPC):
                                nc.tensor.matmul(
                                    ps[:, 0:hi - lo],
                                    attnT[:, h, tt * 128:(tt + 1) * 128],
                                    wo_sb[:, h, lo:hi],
                                    start=(h == 0), stop=(h == HPC - 1))
                            if ci == 2:
                                nc.scalar.copy(out=ob[:, lo:hi],
                                               in_=ps[:, 0:hi - lo])
                            else:
                                nc.vector.tensor_copy(
                                    out=ob[:, lo:hi], in_=ps[:, 0:hi - lo])
                        r = rowblk[tt]
                        nc.sync.dma_start(
                            out=rs_in[r * 128:(r + 1) * 128, :], in_=ob[:])

                    # ---- interleaved A+B+C emission ----
                    for t in range(NT):
                        emit_head(t)
                        if t >= 1:
                            emit_tail(t - 1)
                        if t >= 4 and t % 4 == 0:
                            qb = t // 4 - 1
                            emit_attn(qb)
                            for tt in range(4 * qb, 4 * qb + 4):
                                oproj(tt)
                            if qb == 2:
                                nc.gpsimd.collective_compute(
                                    'ReduceScatter', OP.add,
                                    ins=[rs_in[0:1024, :].opt()],
                                    outs=[rs_out[0:512, :].opt()],
                                    replica_groups=grp)
                    emit_tail(NT - 1)
                    emit_attn(QB - 1)
                    for tt in range(12, 16):
                        oproj(tt)
                    nc.gpsimd.collective_compute(
                        'ReduceScatter', OP.add,
                        ins=[rs_in[1024:2048, :].opt()],
                        outs=[rs_out[512:1024, :].opt()],
                        replica_groups=grp)

            # ====== phase D: residual + MLP over my T/2 tokens ======
            with contextlib.ExitStack() as pd:
                d_t = pd.enter_context(tc.tile_pool(name='d_t', bufs=2))
                d_big = pd.enter_context(tc.tile_pool(name='d_big', bufs=1))
                x2_sb = d_big.tile([128, NH, C], f32)
                xn2T = d_big.tile([128, 10, T // 2], bf16)
                h2T = d_big.tile([128, F // 128, T // 2], bf16)
                rinv_sb = d_big.tile([128, NH], f32)

                with tc.tile_pool(name='d_ps', bufs=4, space='PSUM') as d_ps, \
                     tc.tile_pool(name='dt_ps', bufs=2,
                                  space='PSUM') as dt_ps:
                    for tt in range(NH):
                        rsx = d_t.tile([128, C], bf16, tag='rsx')
                        nc.gpsimd.dma_start(
                            out=rsx[:],
                            in_=rs_out[tt * 128:(tt + 1) * 128, :])
                        xht = d_t.tile([128, C], f32, tag='xht')
                        nc.sync.dma_start(
                            out=xht[:],
                            in_=xh_d[tt * 128:(tt + 1) * 128, :])
                        nc.vector.tensor_add(out=x2_sb[:, tt, :],
                                             in0=rsx[:], in1=xht[:])
                        ssq2 = d_t.tile([128, 1], f32, tag='ssq2')
                        nc.scalar.activation(out=scr_sq[:],
                                             in_=x2_sb[:, tt, :],
                                             func=AF.Square, bias=zero_sb[:],
                                             accum_out=ssq2[:])
                        nc.scalar.activation(out=rinv_sb[:, tt:tt + 1],
                                             in_=ssq2[:], func=AF.Ln,
                                             scale=float(1.0 / C),
                                             bias=eps_sb[:])
                        nc.scalar.activation(out=rinv_sb[:, tt:tt + 1],
                                             in_=rinv_sb[:, tt:tt + 1],
                                             func=AF.Exp, scale=-1.0,
                                             bias=zero_sb[:])
                        for jg, (lo, hi) in enumerate(((0, 4), (4, 8),
                                                      (8, 10))):
                            tp2 = dt_ps.tile([128, 512], f32, tag='tp2')
                            for j in range(lo, hi):
                                nc.tensor.transpose(
                                    tp2[:, (j - lo) * 128:(j - lo + 1) * 128],
                                    x2_sb[:, tt, j * 128:(j + 1) * 128],
                                    ident_f[:])
                            nc.vector.tensor_copy(
                                out=xn2T[:, lo:hi, tt * 128:(tt + 1) * 128],
                                in_=tp2[:, 0:(hi - lo) * 128].rearrange(
                                    'p (j d) -> p j d', d=128))

                    # fc1: j-loop with LDW amortized over two 512 chunks
                    with tc.tile_pool(name='wf1', bufs=3) as wf1_pool:
                        for fi in range(F // 128):
                            wf1 = wf1_pool.tile([128, 10, 128], bf16,
                                                tag='wf1')
                            nc.sync.dma_start(
                                out=wf1[:],
                                in_=wfc1_d[:, fi * 128:(fi + 1) * 128]
                                .rearrange('(j p) c -> p j c', p=128))
                            hp0 = d_ps.tile([128, 512], f32, tag='hps')
                            hp1 = d_ps.tile([128, 512], f32, tag='hps')
                            for j in range(10):
                                nc.tensor.matmul(hp0[:], wf1[:, j, :],
                                                 xn2T[:, j, 0:512],
                                                 start=(j == 0),
                                                 stop=(j == 9))
                                nc.tensor.matmul(hp1[:], wf1[:, j, :],
                                                 xn2T[:, j, 512:1024],
                                                 start=(j == 0),
                                                 stop=(j == 9))
                            for ci, hp in ((0, hp0), (1, hp1)):
                                hrelu = d_t.tile([128, 512], bf16,
                                                 tag='hrelu')
                                nc.scalar.activation(out=hrelu[:], in_=hp[:],
                                                     func=AF.Relu,
                                                     bias=zero_sb[:])
                                nc.vector.tensor_mul(
                                    out=h2T[:, fi,
                                            ci * 512:(ci + 1) * 512],
                                    in0=hrelu[:], in1=hrelu[:])

                # fc2: c-halves x tt-quads; 2 matmuls (640 cols) per lhsT
                with tc.tile_pool(name='y_ps', bufs=4, space='PSUM') as y_ps, \
                     tc.tile_pool(name='wf2', bufs=3) as wf2_pool:
                    for clo, chi in ((0, 640), (640, C)):
                        for ttg in range(2):
                            yps = [y_ps.tile([128, 640], f32, tag='yps',
                                             name='yps')
                                   for _ in range(4)]
                            for fi in range(F // 128):
                                wf2 = wf2_pool.tile([128, 640], bf16,
                                                    tag='wf2')
                                nc.sync.dma_start(
                                    out=wf2[:],
                                    in_=wfc2_d[fi * 128:(fi + 1) * 128,
                                               clo:chi])
                                for i in range(4):
                                    tt = 4 * ttg + i
                                    lhsT = h2T[:, fi,
                                               tt * 128:(tt + 1) * 128]
                                    st = (fi == 0)
                                    sp_ = (fi == F // 128 - 1)
                                    nc.tensor.matmul(yps[i][:, 0:512], lhsT,
                                                     wf2[:, 0:512],
                                                     start=st, stop=sp_)
                                    nc.tensor.matmul(yps[i][:, 512:640],
                                                     lhsT,
                                                     wf2[:, 512:640],
                                                     start=st, stop=sp_)
                            for i in range(4):
                                tt = 4 * ttg + i
                                yo = d_t.tile([128, 640], f32, tag='yo')
                                nc.vector.scalar_tensor_tensor(
                                    out=yo[:], in0=yps[i][:],
                                    scalar=rinv_sb[:, tt:tt + 1],
                                    in1=x2_sb[:, tt, clo:chi],
                                    op0=OP.mult, op1=OP.add)
                                nc.sync.dma_start(
                                    out=y_d[tt * 128:(tt + 1) * 128,
                                            clo:chi],
                                    in_=yo[:])

    nc.compile()
    return nc


_CACHE = {}


def _get_nc(t_len=T):
    if t_len not in _CACHE:
        _CACHE[t_len] = build_nc(t_len)
    return _CACHE[t_len]


def make_in_maps(x, rotary_pos_emb, ln1_w, w_qkv, qn_w, kn_w, w_out, ln2_w,
                 w_fc1, w_fc2, t_len=T):
    """Host-side sharding prep. Returns list of per-core input dicts."""
    x = np.asarray(x, np.float32)
    rot = np.asarray(rotary_pos_emb, np.float32)
    cos = np.cos(rot).astype(np.float32)
    sin = np.sin(rot).astype(np.float32)
    sinneg = np.concatenate([-sin[:, :64], sin[:, :64]], axis=-1)
    qn = np.asarray(qn_w, np.float32)
    kn = np.asarray(kn_w, np.float32)
    cosq = (cos * qn).astype(ml_dtypes.bfloat16)
    sinq = (sinneg * qn).astype(ml_dtypes.bfloat16)
    cosk = (cos * kn).astype(ml_dtypes.bfloat16)
    sink = (sinneg * kn).astype(ml_dtypes.bfloat16)
    w_qkv_f = (np.asarray(w_qkv, np.float32)
               * np.asarray(ln1_w, np.float32)[:, None]).reshape(C, 3, H, D)
    w_fc1_f = (np.asarray(w_fc1, np.float32)
               * np.asarray(ln2_w, np.float32)[:, None]
               ).astype(ml_dtypes.bfloat16)
    w_fc2_b = np.asarray(w_fc2, np.float32).astype(ml_dtypes.bfloat16)
    wo = np.asarray(w_out, np.float32).reshape(H, D, C)

    in_maps = []
    for c in range(N_CORES):
        b, hg = c // 2, c % 2
        heads = slice(hg * HPC, (hg + 1) * HPC)
        wq = np.ascontiguousarray(
            w_qkv_f[:, :, heads, :].reshape(C, 3 * CPC)
        ).astype(ml_dtypes.bfloat16)
        w_outp = np.ascontiguousarray(
            wo[heads].reshape(CPC, C)).astype(ml_dtypes.bfloat16)
        in_maps.append({
            'x': np.ascontiguousarray(x[b]),
            'xh': np.ascontiguousarray(x[b, hg * T // 2:(hg + 1) * T // 2]),
            'w_qkv': wq,
            'cosq': cosq, 'sinq': sinq, 'cosk': cosk, 'sink': sink,
            'w_out': w_outp,
            'w_fc1': np.ascontiguousarray(w_fc1_f),
            'w_fc2': np.ascontiguousarray(w_fc2_b),
        })
    return in_maps


def assemble_output(results, t_len=T):
    out = np.zeros((B, t_len, C), np.float32)
    for c in range(N_CORES):
        b, hg = c // 2, c % 2
        out[b, hg * t_len // 2:(hg + 1) * t_len // 2] = results[c]['y']
    return out


def kernel(**inputs):
    nc = _get_nc(T)
    in_maps = make_in_maps(**inputs)
    res = bass_utils.run_bass_kernel_spmd(nc, in_maps,
                                          core_ids=list(range(N_CORES)))
    return assemble_output(res.results)


# revision 16
# speedup vs baseline: 1.3022x; 1.0143x over previous
"""Trainium2 Bass kernel for nn_Block_27848567948000 (dense transformer block).

Sharding (8 NeuronCores): 4 data-parallel groups over batch (B=4), 2-way
tensor-parallel within each pair: attention sharded over heads (5 each).
out_proj computed as per-head partial sums over ALL T, summed + token-scattered
via a pairwise ReduceScatter; MLP over the core's T/2 token half.

kernel(**inputs) takes FULL inputs and returns the FULL (4, 2048, 1280) output.
"""
import sys

sys.path.insert(0, '/opt/trn_rl_repo')

import numpy as np
import ml_dtypes

import concourse.bass as bass
import concourse.tile as tile
from concourse import mybir, bacc
from concourse import bass_utils
from concourse.masks import make_identity

B, T, C, H, D, F = 4, 2048, 1280, 10, 128, 5120
EPS = 1e-5
N_CORES = 8
HPC = H // 2            # heads per core (5)
CPC = HPC * D           # channels per core (640)
f32 = mybir.dt.float32
bf16 = mybir.dt.bfloat16
fp8 = mybir.dt.float8e4
i32 = mybir.dt.int32
AF = mybir.ActivationFunctionType
OP = mybir.AluOpType
AX = mybir.AxisListType

NT = T // 128            # 16 token tiles
NH = T // 2 // 128       # 8 token tiles in my half
QB = T // 512            # 4 query blocks
NBLK = HPC * 4           # 20 mxfp8 blocks per tensor per token
INV_SQRT_D = float(1.0 / np.sqrt(D))
NEG = -30000.0


def _ap(t_ap, offset_delta, pattern):
    return bass.AP(tensor=t_ap.tensor, offset=t_ap.offset + offset_delta,
                   ap=pattern)


def _rsqrt_vec(nc, pool, out_ap, in_ap, scale, eps, tag):
    """out = 1/sqrt(in*scale + eps) on the vector engine (no act tables).
    Bit-trick seed + 2 Newton iterations (~1e-6 rel err). Shapes (128, n)."""
    i32_ = mybir.dt.int32
    shp = [128, in_ap.free_size()]
    m = pool.tile(shp, f32, tag=tag + 'm', name='rs_m')
    nc.vector.tensor_scalar(out=m[:], in0=in_ap, scalar1=scale, scalar2=eps,
                            op0=OP.mult, op1=OP.add)
    y = pool.tile(shp, f32, tag=tag + 'y', name='rs_y')
    nc.vector.tensor_single_scalar(out=y[:].bitcast(i32_),
                                   in_=m[:].bitcast(i32_), scalar=1,
                                   op=OP.logical_shift_right)
    nc.vector.tensor_scalar(out=y[:].bitcast(i32_), in0=y[:].bitcast(i32_),
                            scalar1=-1, scalar2=0x5f3759df,
                            op0=OP.mult, op1=OP.add)
    t = pool.tile(shp, f32, tag=tag + 't', name='rs_t')
    for _ in range(2):
        nc.vector.tensor_tensor(out=t[:], in0=y[:], in1=y[:], op=OP.mult)
        nc.vector.tensor_tensor(out=t[:], in0=t[:], in1=m[:], op=OP.mult)
        nc.vector.tensor_scalar(out=t[:], in0=t[:], scalar1=-0.5,
                                scalar2=1.5, op0=OP.mult, op1=OP.add)
        nc.vector.tensor_tensor(out=y[:], in0=y[:], in1=t[:], op=OP.mult)
    nc.vector.tensor_copy(out=out_ap, in_=y[:])


def build_nc(t_len=T, n_cores=N_CORES):
    import contextlib
    nc = bacc.Bacc('TRN2', target_bir_lowering=False, debug=False,
                   num_devices=n_cores)

    # ---- DRAM I/O ----
    x_d = nc.dram_tensor('x', [T, C], f32, kind='ExternalInput')
    xh_d = nc.dram_tensor('xh', [T // 2, C], f32, kind='ExternalInput')
    wqkv_d = nc.dram_tensor('w_qkv', [C, 3 * CPC], bf16, kind='ExternalInput')
    cosq_d = nc.dram_tensor('cosq', [T, D], bf16, kind='ExternalInput')
    sinq_d = nc.dram_tensor('sinq', [T, D], bf16, kind='ExternalInput')
    cosk_d = nc.dram_tensor('cosk', [T, D], bf16, kind='ExternalInput')
    sink_d = nc.dram_tensor('sink', [T, D], bf16, kind='ExternalInput')
    wout_d = nc.dram_tensor('w_out', [CPC, C], bf16, kind='ExternalInput')
    wfc1_d = nc.dram_tensor('w_fc1', [C, F], bf16, kind='ExternalInput')
    wfc2_d = nc.dram_tensor('w_fc2', [F, C], bf16, kind='ExternalInput')
    y_d = nc.dram_tensor('y', [T // 2, C], f32, kind='ExternalOutput')

    with tile.TileContext(nc) as tc:
        with contextlib.ExitStack() as ctx:
            persist = ctx.enter_context(tc.tile_pool(name='persist', bufs=1))
            dram = ctx.enter_context(tc.tile_pool(name='dram', bufs=1,
                                                  space='DRAM'))

            # ---- constants ----
            ident_b = persist.tile([128, 128], bf16)
            make_identity(nc, ident_b)
            ident_f = persist.tile([128, 128], f32)
            make_identity(nc, ident_f)
            ones128 = persist.tile([128, 128], bf16)
            nc.vector.memset(ones128[:], 1.0)
            zero_sb = persist.tile([128, 1], f32)
            nc.vector.memset(zero_sb[:], 0.0)
            eps_sb = persist.tile([128, 1], f32)
            nc.vector.memset(eps_sb[:], EPS)
            scr_sq = persist.tile([128, C], bf16)   # Square-output scratch

            # DRAM scratch for the collective
            rs_in = dram.tile([T, C], bf16)
            rs_out = dram.tile([T // 2, C], bf16)

            with contextlib.ExitStack() as pab:
                ab = pab.enter_context(tc.tile_pool(name='ab', bufs=1))
                qT = ab.tile([128, HPC, T], bf16)
                kT = ab.tile([128, HPC, T], bf16)
                vd_sb = ab.tile([128, NT, HPC, 130], bf16)
                attnT = ab.tile([128, HPC, T], bf16)
                nc.vector.memset(vd_sb[:, :, :, 128:129], 1.0)

                # ====== phases A+B ======
                with contextlib.ExitStack() as pin:
                    a_w = pin.enter_context(tc.tile_pool(name='a_w', bufs=1))
                    wq_sb = a_w.tile([128, 10, 3 * CPC], bf16)
                    nc.sync.dma_start(
                        out=wq_sb[:],
                        in_=wqkv_d.ap().rearrange('(j p) c -> p j c', p=128))
                    cq_sb = a_w.tile([128, NT, D], bf16)
                    nc.sync.dma_start(
                        out=cq_sb[:],
                        in_=cosq_d.ap().rearrange('(t p) d -> p t d', p=128))
                    sq_sb = a_w.tile([128, NT, D], bf16)
                    nc.sync.dma_start(
                        out=sq_sb[:],
                        in_=sinq_d.ap().rearrange('(t p) d -> p t d', p=128))
                    ck_sb = a_w.tile([128, NT, D], bf16)
                    nc.sync.dma_start(
                        out=ck_sb[:],
                        in_=cosk_d.ap().rearrange('(t p) d -> p t d', p=128))
                    sk_sb = a_w.tile([128, NT, D], bf16)
                    nc.sync.dma_start(
                        out=sk_sb[:],
                        in_=sink_d.ap().rearrange('(t p) d -> p t d', p=128))

                    a_t = pin.enter_context(tc.tile_pool(name='a_t', bufs=2))
                    a_s = pin.enter_context(tc.tile_pool(name='a_s', bufs=2))
                    a_q = pin.enter_context(tc.tile_pool(name='a_q', bufs=2))
                    pT_pool = pin.enter_context(
                        tc.tile_pool(name='pT', bufs=3))
                    b_t = pin.enter_context(tc.tile_pool(name='b_t', bufs=2))
                    ps512 = pin.enter_context(
                        tc.tile_pool(name='ps512', bufs=3, space='PSUM'))
                    ops_ps = pin.enter_context(
                        tc.tile_pool(name='ops_ps', bufs=1, space='PSUM'))
                    psT = pin.enter_context(
                        tc.tile_pool(name='psT', bufs=3, space='PSUM'))
                    psD = pin.enter_context(
                        tc.tile_pool(name='psD', bufs=1, space='PSUM'))

                    stash = {}

                    def emit_head(t):
                        xt = a_s.tile([128, C], f32, tag='xt')
                        nc.sync.dma_start(out=xt[:],
                                          in_=x_d[t * 128:(t + 1) * 128, :])
                        ssq = a_s.tile([128, 1], f32, tag='ssq')
                        nc.scalar.activation(out=scr_sq[:], in_=xt[:],
                                             func=AF.Square, bias=zero_sb[:],
                                             accum_out=ssq[:])
                        rstd = a_s.tile([128, 1], f32, tag='rstd')
                        _rsqrt_vec(nc, a_s, rstd[:], ssq[:],
                                   float(1.0 / C), EPS, 'rx')
                        xnT = a_s.tile([128, 10, 128], bf16, tag='xnT')
                        for jg, (lo, hi) in enumerate(((0, 4), (4, 8),
                                                      (8, 10))):
                            tp = psT.tile([128, 512], f32, tag='tp',
                                          name='tpf')
                            for j in range(lo, hi):
                                nc.tensor.transpose(
                                    tp[:, (j - lo) * 128:(j - lo + 1) * 128],
                                    xt[:, j * 128:(j + 1) * 128], ident_f[:])
                            nc.vector.tensor_copy(
                                out=xnT[:, lo:hi, :],
                                in_=tp[:, 0:(hi - lo) * 128].rearrange(
                                    'p (j d) -> p j d', d=128))
                        # QKV (chunk-outer, j-mid, g-inner: LDW amortized)
                        qf = a_q.tile([128, CPC], bf16, tag='qf')
                        kf = a_q.tile([128, CPC], bf16, tag='kf')
                        vf = a_q.tile([128, CPC], bf16, tag='vf')
                        dsts = (qf, kf, vf)
                        for lo, hi in ((0, 512), (512, 640)):
                            pss = [ps512.tile([128, 512], f32, tag='mm',
                                              name='qkvps')
                                   for _ in range(3)]
                            for j in range(10):
                                for g in range(3):
                                    nc.tensor.matmul(
                                        pss[g][:, 0:hi - lo], xnT[:, j, :],
                                        wq_sb[:, j,
                                              g * CPC + lo:g * CPC + hi],
                                        start=(j == 0), stop=(j == 9))
                            for g in range(3):
                                if g == 1:
                                    nc.scalar.activation(
                                        out=dsts[g][:, lo:hi],
                                        in_=pss[g][:, 0:hi - lo],
                                        func=AF.Copy, scale=rstd[:])
                                else:
                                    nc.vector.tensor_scalar_mul(
                                        out=dsts[g][:, lo:hi],
                                        in0=pss[g][:, 0:hi - lo],
                                        scalar1=rstd[:])
                        stash[t] = (qf, kf, vf)

                    def rope(eng, src, cos_t, sin_t, out):
                        # out[p,h,d] = src*cos + swap(src)*sinneg   (bf16)
                        src3 = src[:].rearrange('p (h d) -> p h d', h=HPC)
                        pa = list(src3.ap)
                        swap = _ap(src3, 64, pa[:2] + [[-64, 2], [1, 64]])
                        ca = list(cos_t.ap)
                        cos4 = _ap(cos_t, 0, [ca[0], [0, HPC], [1, 128]])
                        sin4 = _ap(sin_t, 0,
                                   [ca[0], [0, HPC], [64, 2], [1, 64]])
                        tmp = a_t.tile([128, HPC, D], bf16, tag='rtmp')
                        eng.tensor_tensor(
                            out=tmp[:].rearrange('p h (u d) -> p h u d', u=2),
                            in0=swap, in1=sin4, op=OP.mult)
                        eng.tensor_tensor(out=out[:], in0=src3, in1=cos4,
                                          op=OP.mult)
                        eng.tensor_add(out=out[:], in0=out[:], in1=tmp[:])

                    def blk4(ap20, reps=32):
                        # (128,20) -> (128,5,4,reps) block broadcast
                        a = list(ap20.ap)
                        st = a[-1][0]
                        return bass.AP(tensor=ap20.tensor, offset=ap20.offset,
                                       ap=[a[0], [4 * st, HPC], [st, 4],
                                           [0, reps]])

                    def hb(ap5, reps=4):
                        # (128,5) -> (128,5,reps) broadcast
                        a = list(ap5.ap)
                        return bass.AP(tensor=ap5.tensor, offset=ap5.offset,
                                       ap=[a[0], [a[-1][0], HPC], [0, reps]])

                    def v4(x):
                        return x.rearrange('p h (b e) -> p h b e', e=32)

                    def emit_tail(t):
                        qf, kf, vf = stash.pop(t)
                        # rms of pre-rope q/k (rope is norm-preserving)
                        msq = a_t.tile([128, 2, HPC], f32, tag='msq')
                        for h in range(HPC):
                            nc.scalar.activation(
                                out=scr_sq[:, 0:D],
                                in_=qf[:, h * D:(h + 1) * D],
                                func=AF.Square, bias=zero_sb[:],
                                accum_out=msq[:, 0, h:h + 1])
                        ksq = a_t.tile([128, HPC, D], bf16, tag='ksq')
                        kf3 = kf[:].rearrange('p (h d) -> p h d', h=HPC)
                        nc.vector.tensor_tensor(out=ksq[:], in0=kf3,
                                                in1=kf3, op=OP.mult)
                        nc.vector.tensor_reduce(out=msq[:, 1, :],
                                                in_=ksq[:], axis=AX.X,
                                                op=OP.add)
                        _rsqrt_vec(nc, a_t, msq[:], msq[:],
                                   float(1.0 / D), EPS, 'rqk')
                        # rope (q on vector, k on gpsimd)
                        zq = a_t.tile([128, HPC, D], bf16, tag='zq')
                        rope(nc.vector, qf, cq_sb[:, t, :], sq_sb[:, t, :],
                             zq)
                        zk = a_t.tile([128, HPC, D], bf16, tag='zk')
                        rope(nc.gpsimd, kf, ck_sb[:, t, :], sk_sb[:, t, :],
                             zk)
                        # block amax; amn = amax*rstd (q,k) or amax (v)
                        amn = a_t.tile([128, 3, NBLK], f32, tag='amn')
                        nc.vector.tensor_reduce(
                            out=amn[:, 0, :], in_=v4(zq[:]), axis=AX.X,
                            op=OP.max, apply_absolute_value=True)
                        nc.vector.tensor_reduce(
                            out=amn[:, 1, :], in_=v4(zk[:]), axis=AX.X,
                            op=OP.max, apply_absolute_value=True)
                        nc.vector.tensor_reduce(
                            out=amn[:, 2, :],
                            in_=vf[:].rearrange('p (h b e) -> p h b e',
                                                h=HPC, e=32),
                            axis=AX.X, op=OP.max, apply_absolute_value=True)
                        for i in range(2):
                            nc.vector.tensor_tensor(
                                out=amn[:, i, :].rearrange(
                                    'p (h b) -> p h b', h=HPC),
                                in0=amn[:, i, :].rearrange(
                                    'p (h b) -> p h b', h=HPC),
                                in1=hb(msq[:, i, :]), op=OP.mult)
                        nc.vector.tensor_scalar_max(out=amn[:], in0=amn[:],
                                                    scalar1=1e-12)
                        eb = a_t.tile([128, 3, NBLK], i32, tag='eb')
                        nc.vector.tensor_single_scalar(
                            out=eb[:], in_=amn[:].bitcast(i32), scalar=23,
                            op=OP.logical_shift_right)
                        sc = a_t.tile([128, 3, NBLK], f32, tag='sc')
                        nc.vector.tensor_scalar(
                            out=sc[:].bitcast(i32), in0=eb[:], scalar1=-1,
                            scalar2=260, op0=OP.mult, op1=OP.add)
                        nc.vector.tensor_single_scalar(
                            out=sc[:].bitcast(i32), in_=sc[:].bitcast(i32),
                            scalar=23, op=OP.logical_shift_left)
                        isc = a_t.tile([128, 3, NBLK], f32, tag='isc')
                        nc.vector.tensor_single_scalar(
                            out=isc[:].bitcast(i32), in_=eb[:], scalar=6,
                            op=OP.subtract)
                        nc.vector.tensor_single_scalar(
                            out=isc[:].bitcast(i32), in_=isc[:].bitcast(i32),
                            scalar=23, op=OP.logical_shift_left)
                        msc = a_t.tile([128, 2, NBLK], f32, tag='msc')
                        for i in range(2):
                            nc.vector.tensor_tensor(
                                out=msc[:, i, :].rearrange(
                                    'p (h b) -> p h b', h=HPC),
                                in0=sc[:, i, :].rearrange(
                                    'p (h b) -> p h b', h=HPC),
                                in1=hb(msq[:, i, :]), op=OP.mult)
                        # quantize q (vector)
                        ys = a_t.tile([128, HPC, D], bf16, tag='ys')
                        q8 = a_t.tile([128, HPC, D], fp8, tag='q8')
                        qd = a_t.tile([128, HPC, D], bf16, tag='qd')
                        nc.vector.tensor_tensor(out=v4(ys[:]), in0=v4(zq[:]),
                                                in1=blk4(msc[:, 0, :]),
                                                op=OP.mult)
                        nc.vector.tensor_scalar(out=q8[:], in0=ys[:],
                                                scalar1=-112.0,
                                                scalar2=112.0,
                                                op0=OP.max, op1=OP.min)
                        nc.vector.tensor_tensor(out=v4(qd[:]), in0=v4(q8[:]),
                                                in1=blk4(isc[:, 0, :]),
                                                op=OP.mult)
                        # quantize k (gpsimd mults, vector fp8 cast)
                        ysk = a_t.tile([128, HPC, D], bf16, tag='ys')
                        k8 = a_t.tile([128, HPC, D], fp8, tag='q8')
                        kd = a_t.tile([128, HPC, D], bf16, tag='kd')
                        nc.gpsimd.tensor_tensor(out=v4(ysk[:]),
                                                in0=v4(zk[:]),
                                                in1=blk4(msc[:, 1, :]),
                                                op=OP.mult)
                        nc.vector.tensor_scalar(out=k8[:], in0=ysk[:],
                                                scalar1=-112.0,
                                                scalar2=112.0,
                                                op0=OP.max, op1=OP.min)
                        nc.gpsimd.tensor_tensor(out=v4(kd[:]), in0=v4(k8[:]),
                                                in1=blk4(isc[:, 1, :]),
                                                op=OP.mult)
                        # quantize v (vector; deq straight into vd_sb)
                        ysv = a_t.tile([128, HPC, D], bf16, tag='ys')
                        v8 = a_t.tile([128, HPC, D], fp8, tag='q8')
                        nc.vector.tensor_tensor(
                            out=v4(ysv[:]),
                            in0=v4(vf[:].rearrange('p (h d) -> p h d',
                                                   h=HPC)),
                            in1=blk4(sc[:, 2, :]), op=OP.mult)
                        nc.vector.tensor_scalar(out=v8[:], in0=ysv[:],
                                                scalar1=-112.0,
                                                scalar2=112.0,
                                                op0=OP.max, op1=OP.min)
                        nc.vector.tensor_tensor(
                            out=v4(vd_sb[:, t, :, 0:D]), in0=v4(v8[:]),
                            in1=blk4(isc[:, 2, :]), op=OP.mult)
                        # transpose qd/kd into qT/kT
                        for src, dstT in ((qd, qT), (kd, kT)):
                            tp = psT.tile([128, 640], bf16, tag='tp')
                            for h in range(HPC):
                                nc.tensor.transpose(
                                    tp[:, h * 128:(h + 1) * 128],
                                    src[:, h, :], ident_b[:])
                            nc.vector.tensor_copy(
                                out=dstT[:, :, t * 128:(t + 1) * 128],
                                in_=tp[:].rearrange('p (h d) -> p h d',
                                                    h=HPC))

                    def emit_attn(qb):
                        nkt = 4 * qb + 4
                        for h in range(HPC):
                            dps = psD.tile([128, 512], f32, tag='dps')
                            ops = ops_ps.tile([128, 512], f32, tag='ops')
                            for kt in range(nkt):
                                sp = ps512.tile([128, 512], f32, tag='mm')
                                o = kt - 4 * qb
                                nc.tensor.matmul(
                                    sp[:],
                                    kT[:, h, kt * 128:(kt + 1) * 128],
                                    qT[:, h, qb * 512:(qb + 1) * 512],
                                    start=True, stop=True)
                                pT = pT_pool.tile([128, 512], bf16, tag='pT')
                                nc.scalar.activation(out=pT[:], in_=sp[:],
                                                     func=AF.Exp,
                                                     bias=zero_sb[:],
                                                     scale=INV_SQRT_D)
                                if o >= 0:
                                    nc.gpsimd.affine_select(
                                        out=pT[:], in_=pT[:],
                                        compare_op=OP.is_ge, fill=0.0,
                                        base=-128 * o, pattern=[[1, 512]],
                                        channel_multiplier=-1)
                                nc.tensor.matmul(dps[:], ones128[:], pT[:],
                                                 start=(kt == 0),
                                                 stop=(kt == nkt - 1))
                                nc.tensor.matmul(ops[:],
                                                 vd_sb[:, kt, h, 0:128],
                                                 pT[:],
                                                 start=(kt == 0),
                                                 stop=(kt == nkt - 1))
                            rd = b_t.tile([128, 512], f32, tag='rd')
                            nc.vector.reciprocal_approx_fast(out=rd[:],
                                                             in_=dps[:])
                            nc.vector.tensor_tensor(
                                out=attnT[:, h, qb * 512:(qb + 1) * 512],
                                in0=ops[:], in1=rd[:], op=OP.mult)

                    wo_sb = a_w.tile([128, HPC, C], bf16)
                    nc.sync.dma_start(
                        out=wo_sb[:],
                        in_=wout_d.ap().rearrange('(h p) c -> p h c', p=128))
                    # rs_in row layout: [t0:512 | t1024:1536 | t512:1024
                    # | t1536:2048] so each RS half is contiguous.
                    rowblk = {tt: i for i, tt in enumerate(
                        (0, 1, 2, 3, 8, 9, 10, 11, 4, 5, 6, 7,
                         12, 13, 14, 15))}
                    grp = [[2 * i, 2 * i + 1] for i in range(n_cores // 2)]

                    def oproj(tt):
                        ob = b_t.tile([128, C], bf16, tag='ob', name='ob')
                        for ci, (lo, hi) in enumerate(((0, 512),
                                                       (512, 1024),
                                                       (1024, C))):
                            ps = ps512.tile([128, 512], f32, tag='mm',
                                            name='oprojps')
                            for h in range(HPC):
                                nc.tensor.matmul(
                                    ps[:, 0:hi - lo],
                                    attnT[:, h, tt * 128:(tt + 1) * 128],
                                    wo_sb[:, h, lo:hi],
                                    start=(h == 0), stop=(h == HPC - 1))
                            if ci == 2:
                                nc.scalar.copy(out=ob[:, lo:hi],
                                               in_=ps[:, 0:hi - lo])
                            else:
                                nc.vector.tensor_copy(
                                    out=ob[:, lo:hi], in_=ps[:, 0:hi - lo])
                        r = rowblk[tt]
                        nc.sync.dma_start(
                            out=rs_in[r * 128:(r + 1) * 128, :], in_=ob[:])

                    # ---- interleaved A+B+C emission ----
                    for t in range(NT):
                        emit_head(t)
                        if t >= 1:
                            emit_tail(t - 1)
                        if t >= 4 and t % 4 == 0:
                            qb = t // 4 - 1
                            emit_attn(qb)
                            for tt in range(4 * qb, 4 * qb + 4):
                                oproj(tt)
                            if qb == 2:
                                nc.gpsimd.collective_compute(
                                    'ReduceScatter', OP.add,
                                    ins=[rs_in[0:1024, :].opt()],
                                    outs=[rs_out[0:512, :].opt()],
                                    replica_groups=grp)
                    emit_tail(NT - 1)
                    emit_attn(QB - 1)
                    for tt in range(12, 16):
                        oproj(tt)
                    nc.gpsimd.collective_compute(
                        'ReduceScatter', OP.add,
                        ins=[rs_in[1024:2048, :].opt()],
                        outs=[rs_out[512:1024, :].opt()],
                        replica_groups=grp)

            # ====== phase D: residual + MLP over my T/2 tokens ======
            with contextlib.ExitStack() as pd:
                d_t = pd.enter_context(tc.tile_pool(name='d_t', bufs=2))
                d_big = pd.enter_context(tc.tile_pool(name='d_big', bufs=1))
                x2_sb = d_big.tile([128, NH, C], f32)
                xn2T = d_big.tile([128, 10, T // 2], bf16)
                h2T = d_big.tile([128, F // 128, T // 2], bf16)
                rinv_sb = d_big.tile([128, NH], f32)

                with tc.tile_pool(name='d_ps', bufs=4, space='PSUM') as d_ps, \
                     tc.tile_pool(name='dt_ps', bufs=2,
                                  space='PSUM') as dt_ps:
                    for tt in range(NH):
                        rsx = d_t.tile([128, C], bf16, tag='rsx')
                        nc.gpsimd.dma_start(
                            out=rsx[:],
                            in_=rs_out[tt * 128:(tt + 1) * 128, :])
                        xht = d_t.tile([128, C], f32, tag='xht')
                        nc.sync.dma_start(
                            out=xht[:],
                            in_=xh_d[tt * 128:(tt + 1) * 128, :])
                        nc.vector.tensor_add(out=x2_sb[:, tt, :],
                                             in0=rsx[:], in1=xht[:])
                        ssq2 = d_t.tile([128, 1], f32, tag='ssq2')
                        nc.scalar.activation(out=scr_sq[:],
                                             in_=x2_sb[:, tt, :],
                                             func=AF.Square, bias=zero_sb[:],
                                             accum_out=ssq2[:])
                        m2 = d_t.tile([128, 1], f32, tag='m2')
                        nc.vector.tensor_scalar(out=m2[:], in0=ssq2[:],
                                                scalar1=float(1.0 / C),
                                                scalar2=EPS,
                                                op0=OP.mult, op1=OP.add)
                        nc.vector.reciprocal_approx_fast(
                            out=rinv_sb[:, tt:tt + 1], in_=m2[:])
                        for jg, (lo, hi) in enumerate(((0, 4), (4, 8),
                                                      (8, 10))):
                            tp2 = dt_ps.tile([128, 512], f32, tag='tp2')
                            for j in range(lo, hi):
                                nc.tensor.transpose(
                                    tp2[:, (j - lo) * 128:(j - lo + 1) * 128],
                                    x2_sb[:, tt, j * 128:(j + 1) * 128],
                                    ident_f[:])
                            nc.vector.tensor_copy(
                                out=xn2T[:, lo:hi, tt * 128:(tt + 1) * 128],
                                in_=tp2[:, 0:(hi - lo) * 128].rearrange(
                                    'p (j d) -> p j d', d=128))

                    # fc1: j-loop with LDW amortized over two 512 chunks
                    with tc.tile_pool(name='wf1', bufs=5) as wf1_pool:
                        for fi in range(F // 128):
                            wf1 = wf1_pool.tile([128, 10, 128], bf16,
                                                tag='wf1')
                            nc.sync.dma_start(
                                out=wf1[:],
                                in_=wfc1_d[:, fi * 128:(fi + 1) * 128]
                                .rearrange('(j p) c -> p j c', p=128))
                            hp0 = d_ps.tile([128, 512], f32, tag='hps')
                            hp1 = d_ps.tile([128, 512], f32, tag='hps')
                            for j in range(10):
                                nc.tensor.matmul(hp0[:], wf1[:, j, :],
                                                 xn2T[:, j, 0:512],
                                                 start=(j == 0),
                                                 stop=(j == 9))
                                nc.tensor.matmul(hp1[:], wf1[:, j, :],
                                                 xn2T[:, j, 512:1024],
                                                 start=(j == 0),
                                                 stop=(j == 9))
                            for ci, hp in ((0, hp0), (1, hp1)):
                                hrelu = d_t.tile([128, 512], bf16,
                                                 tag='hrelu')
                                nc.scalar.activation(out=hrelu[:], in_=hp[:],
                                                     func=AF.Relu,
                                                     bias=zero_sb[:])
                                nc.vector.tensor_mul(
                                    out=h2T[:, fi,
                                            ci * 512:(ci + 1) * 512],
                                    in0=hrelu[:], in1=hrelu[:])

                # fc2: c-halves x tt-quads; 2 matmuls (640 cols) per lhsT
                with tc.tile_pool(name='y_ps', bufs=4, space='PSUM') as y_ps, \
                     tc.tile_pool(name='wf2', bufs=3) as wf2_pool:
                    for clo, chi in ((0, 640), (640, C)):
                        for ttg in range(2):
                            yps = [y_ps.tile([128, 640], f32, tag='yps',
                                             name='yps')
                                   for _ in range(4)]
                            for f2 in range(F // 256):
                                wf2 = wf2_pool.tile([128, 2, 640], bf16,
                                                    tag='wf2')
                                nc.sync.dma_start(
                                    out=wf2[:],
                                    in_=wfc2_d[f2 * 256:(f2 + 1) * 256,
                                               clo:chi]
                                    .rearrange('(u p) c -> p u c', p=128))
                                for u in range(2):
                                    fi = 2 * f2 + u
                                    st = (fi == 0)
                                    sp_ = (fi == F // 128 - 1)
                                    for i in range(4):
                                        tt = 4 * ttg + i
                                        lhsT = h2T[:, fi,
                                                   tt * 128:(tt + 1) * 128]
                                        nc.tensor.matmul(
                                            yps[i][:, 0:512], lhsT,
                                            wf2[:, u, 0:512],
                                            start=st, stop=sp_)
                                        nc.tensor.matmul(
                                            yps[i][:, 512:640], lhsT,
                                            wf2[:, u, 512:640],
                                            start=st, stop=sp_)
                            for i in range(4):
                                tt = 4 * ttg + i
                                yo = d_t.tile([128, 640], f32, tag='yo')
                                nc.vector.scalar_tensor_tensor(
                                    out=yo[:], in0=yps[i][:],
                                    scalar=rinv_sb[:, tt:tt + 1],
                                    in1=x2_sb[:, tt, clo:chi],
                                    op0=OP.mult, op1=OP.add)
                                nc.sync.dma_start(
                                    out=y_d[tt * 128:(tt + 1) * 128,
                                            clo:chi],
                                    in_=yo[:])

    nc.compile()
    return nc


_CACHE = {}


def _get_nc(t_len=T):
    if t_len not in _CACHE:
        _CACHE[t_len] = build_nc(t_len)
    return _CACHE[t_len]


def make_in_maps(x, rotary_pos_emb, ln1_w, w_qkv, qn_w, kn_w, w_out, ln2_w,
                 w_fc1, w_fc2, t_len=T):
    """Host-side sharding prep. Returns list of per-core input dicts."""
    x = np.asarray(x, np.float32)
    rot = np.asarray(rotary_pos_emb, np.float32)
    cos = np.cos(rot).astype(np.float32)
    sin = np.sin(rot).astype(np.float32)
    sinneg = np.concatenate([-sin[:, :64], sin[:, :64]], axis=-1)
    qn = np.asarray(qn_w, np.float32)
    kn = np.asarray(kn_w, np.float32)
    cosq = (cos * qn).astype(ml_dtypes.bfloat16)
    sinq = (sinneg * qn).astype(ml_dtypes.bfloat16)
    cosk = (cos * kn).astype(ml_dtypes.bfloat16)
    sink = (sinneg * kn).astype(ml_dtypes.bfloat16)
    w_qkv_f = (np.asarray(w_qkv, np.float32)
               * np.asarray(ln1_w, np.float32)[:, None]).reshape(C, 3, H, D)
    w_fc1_f = (np.asarray(w_fc1, np.float32)
               * np.asarray(ln2_w, np.float32)[:, None]
               ).astype(ml_dtypes.bfloat16)
    w_fc2_b = np.asarray(w_fc2, np.float32).astype(ml_dtypes.bfloat16)
    wo = np.asarray(w_out, np.float32).reshape(H, D, C)

    in_maps = []
    for c in range(N_CORES):
        b, hg = c // 2, c % 2
        heads = slice(hg * HPC, (hg + 1) * HPC)
        wq = np.ascontiguousarray(
            w_qkv_f[:, :, heads, :].reshape(C, 3 * CPC)
        ).astype(ml_dtypes.bfloat16)
        w_outp = np.ascontiguousarray(
            wo[heads].reshape(CPC, C)).astype(ml_dtypes.bfloat16)
        in_maps.append({
            'x': np.ascontiguousarray(x[b]),
            'xh': np.ascontiguousarray(x[b, hg * T // 2:(hg + 1) * T // 2]),
            'w_qkv': wq,
            'cosq': cosq, 'sinq': sinq, 'cosk': cosk, 'sink': sink,
            'w_out': w_outp,
            'w_fc1': np.ascontiguousarray(w_fc1_f),
            'w_fc2': np.ascontiguousarray(w_fc2_b),
        })
    return in_maps


def assemble_output(results, t_len=T):
    out = np.zeros((B, t_len, C), np.float32)
    for c in range(N_CORES):
        b, hg = c // 2, c % 2
        out[b, hg * t_len // 2:(hg + 1) * t_len // 2] = results[c]['y']
    return out


def kernel(**inputs):
    nc = _get_nc(T)
    in_maps = make_in_maps(**inputs)
    res = bass_utils.run_bass_kernel_spmd(nc, in_maps,
                                          core_ids=list(range(N_CORES)))
    return assemble_output(res.results)


# revision 19
# speedup vs baseline: 1.3700x; 1.0521x over previous
"""Trainium2 Bass kernel for nn_Block_27848567948000 (dense transformer block).

Sharding (8 NeuronCores): 4 data-parallel groups over batch (B=4), 2-way
tensor-parallel within each pair: attention sharded over heads (5 each).
out_proj computed as per-head partial sums over ALL T, summed + token-scattered
via a pairwise ReduceScatter; MLP over the core's T/2 token half.

kernel(**inputs) takes FULL inputs and returns the FULL (4, 2048, 1280) output.
"""
import sys

sys.path.insert(0, '/opt/trn_rl_repo')

import numpy as np
import ml_dtypes

import concourse.bass as bass
import concourse.tile as tile
from concourse import mybir, bacc
from concourse import bass_utils
from concourse.masks import make_identity

B, T, C, H, D, F = 4, 2048, 1280, 10, 128, 5120
EPS = 1e-5
N_CORES = 8
HPC = H // 2            # heads per core (5)
CPC = HPC * D           # channels per core (640)
f32 = mybir.dt.float32
bf16 = mybir.dt.bfloat16
fp8 = mybir.dt.float8e4
i32 = mybir.dt.int32
AF = mybir.ActivationFunctionType
OP = mybir.AluOpType
AX = mybir.AxisListType

NT = T // 128            # 16 token tiles
NH = T // 2 // 128       # 8 token tiles in my half
QB = T // 512            # 4 query blocks
NBLK = HPC * 4           # 20 mxfp8 blocks per tensor per token
INV_SQRT_D = float(1.0 / np.sqrt(D))
NEG = -30000.0


def _ap(t_ap, offset_delta, pattern):
    return bass.AP(tensor=t_ap.tensor, offset=t_ap.offset + offset_delta,
                   ap=pattern)


def _rsqrt_vec(nc, pool, out_ap, in_ap, scale, eps, tag, eng=None):
    """out = 1/sqrt(in*scale + eps) on a DVE-like engine (no act tables).
    Bit-trick seed + 2 Newton iterations (~1e-6 rel err). Shapes (128, n)."""
    if eng is None:
        eng = nc.vector
    i32_ = mybir.dt.int32
    shp = [128, in_ap.free_size()]
    m = pool.tile(shp, f32, tag=tag + 'm', name='rs_m')
    eng.tensor_scalar(out=m[:], in0=in_ap, scalar1=scale, scalar2=eps,
                      op0=OP.mult, op1=OP.add)
    y = pool.tile(shp, f32, tag=tag + 'y', name='rs_y')
    eng.tensor_single_scalar(out=y[:].bitcast(i32_),
                             in_=m[:].bitcast(i32_), scalar=1,
                             op=OP.logical_shift_right)
    eng.tensor_scalar(out=y[:].bitcast(i32_), in0=y[:].bitcast(i32_),
                      scalar1=-1, scalar2=0x5f3759df,
                      op0=OP.mult, op1=OP.add)
    t = pool.tile(shp, f32, tag=tag + 't', name='rs_t')
    for it in range(2):
        eng.tensor_tensor(out=t[:], in0=y[:], in1=y[:], op=OP.mult)
        eng.tensor_tensor(out=t[:], in0=t[:], in1=m[:], op=OP.mult)
        eng.tensor_scalar(out=t[:], in0=t[:], scalar1=-0.5,
                          scalar2=1.5, op0=OP.mult, op1=OP.add)
        eng.tensor_tensor(out=y[:] if it == 0 else out_ap, in0=y[:],
                          in1=t[:], op=OP.mult)


def build_nc(t_len=T, n_cores=N_CORES):
    import contextlib
    nc = bacc.Bacc('TRN2', target_bir_lowering=False, debug=False,
                   num_devices=n_cores)

    # ---- DRAM I/O ----
    x_d = nc.dram_tensor('x', [T, C], f32, kind='ExternalInput')
    xh_d = nc.dram_tensor('xh', [T // 2, C], f32, kind='ExternalInput')
    wqkv_d = nc.dram_tensor('w_qkv', [C, 3 * CPC], bf16, kind='ExternalInput')
    cosq_d = nc.dram_tensor('cosq', [T, D], bf16, kind='ExternalInput')
    sinq_d = nc.dram_tensor('sinq', [T, D], bf16, kind='ExternalInput')
    cosk_d = nc.dram_tensor('cosk', [T, D], bf16, kind='ExternalInput')
    sink_d = nc.dram_tensor('sink', [T, D], bf16, kind='ExternalInput')
    wout_d = nc.dram_tensor('w_out', [CPC, C], bf16, kind='ExternalInput')
    wfc1_d = nc.dram_tensor('w_fc1', [C, F], bf16, kind='ExternalInput')
    wfc2_d = nc.dram_tensor('w_fc2', [F, C], bf16, kind='ExternalInput')
    y_d = nc.dram_tensor('y', [T // 2, C], f32, kind='ExternalOutput')

    with tile.TileContext(nc) as tc:
        with contextlib.ExitStack() as ctx:
            persist = ctx.enter_context(tc.tile_pool(name='persist', bufs=1))
            dram = ctx.enter_context(tc.tile_pool(name='dram', bufs=1,
                                                  space='DRAM'))

            # ---- constants ----
            ident_b = persist.tile([128, 128], bf16)
            make_identity(nc, ident_b)
            ident_f = persist.tile([128, 128], f32)
            make_identity(nc, ident_f)
            ones128 = persist.tile([128, 128], bf16)
            nc.vector.memset(ones128[:], 1.0)
            zero_sb = persist.tile([128, 1], f32)
            nc.vector.memset(zero_sb[:], 0.0)
            eps_sb = persist.tile([128, 1], f32)
            nc.vector.memset(eps_sb[:], EPS)
            scr_sq = persist.tile([128, C], bf16)   # Square-output scratch

            # DRAM scratch for the collective
            rs_in = dram.tile([T, C], bf16)
            rs_out = dram.tile([T // 2, C], bf16)

            with contextlib.ExitStack() as pab:
                ab = pab.enter_context(tc.tile_pool(name='ab', bufs=1))
                qT = ab.tile([128, HPC, T], bf16)
                kT = ab.tile([128, HPC, T], bf16)
                vd_sb = ab.tile([128, NT, HPC, 130], bf16)
                attnT = ab.tile([128, HPC, T], bf16)
                nc.vector.memset(vd_sb[:, :, :, 128:129], 1.0)

                # ====== phases A+B ======
                with contextlib.ExitStack() as pin:
                    a_w = pin.enter_context(tc.tile_pool(name='a_w', bufs=1))
                    wq_sb = a_w.tile([128, 10, 3 * CPC], bf16)
                    nc.sync.dma_start(
                        out=wq_sb[:],
                        in_=wqkv_d.ap().rearrange('(j p) c -> p j c', p=128))
                    cq_sb = a_w.tile([128, NT, D], bf16)
                    nc.sync.dma_start(
                        out=cq_sb[:],
                        in_=cosq_d.ap().rearrange('(t p) d -> p t d', p=128))
                    sq_sb = a_w.tile([128, NT, D], bf16)
                    nc.sync.dma_start(
                        out=sq_sb[:],
                        in_=sinq_d.ap().rearrange('(t p) d -> p t d', p=128))
                    ck_sb = a_w.tile([128, NT, D], bf16)
                    nc.sync.dma_start(
                        out=ck_sb[:],
                        in_=cosk_d.ap().rearrange('(t p) d -> p t d', p=128))
                    sk_sb = a_w.tile([128, NT, D], bf16)
                    nc.sync.dma_start(
                        out=sk_sb[:],
                        in_=sink_d.ap().rearrange('(t p) d -> p t d', p=128))

                    a_t = pin.enter_context(tc.tile_pool(name='a_t', bufs=2))
                    a_s = pin.enter_context(tc.tile_pool(name='a_s', bufs=2))
                    a_q = pin.enter_context(tc.tile_pool(name='a_q', bufs=2))
                    pT_pool = pin.enter_context(
                        tc.tile_pool(name='pT', bufs=4))
                    b_t = pin.enter_context(tc.tile_pool(name='b_t', bufs=2))
                    ps512 = pin.enter_context(
                        tc.tile_pool(name='ps512', bufs=3, space='PSUM'))
                    ops_ps = pin.enter_context(
                        tc.tile_pool(name='ops_ps', bufs=1, space='PSUM'))
                    psT = pin.enter_context(
                        tc.tile_pool(name='psT', bufs=3, space='PSUM'))
                    psD = pin.enter_context(
                        tc.tile_pool(name='psD', bufs=1, space='PSUM'))

                    stash = {}

                    def emit_head(t):
                        xt = a_s.tile([128, C], f32, tag='xt')
                        nc.sync.dma_start(out=xt[:],
                                          in_=x_d[t * 128:(t + 1) * 128, :])
                        ssq = a_s.tile([128, 1], f32, tag='ssq')
                        nc.scalar.activation(out=scr_sq[:], in_=xt[:],
                                             func=AF.Square, bias=zero_sb[:],
                                             accum_out=ssq[:])
                        rstd = a_s.tile([128, 1], f32, tag='rstd')
                        _rsqrt_vec(nc, a_s, rstd[:], ssq[:],
                                   float(1.0 / C), EPS, 'rx')
                        xnT = a_s.tile([128, 10, 128], bf16, tag='xnT')
                        for jg, (lo, hi) in enumerate(((0, 4), (4, 8),
                                                      (8, 10))):
                            tp = psT.tile([128, 512], f32, tag='tp',
                                          name='tpf')
                            for j in range(lo, hi):
                                nc.tensor.transpose(
                                    tp[:, (j - lo) * 128:(j - lo + 1) * 128],
                                    xt[:, j * 128:(j + 1) * 128], ident_f[:])
                            nc.scalar.copy(
                                out=xnT[:, lo:hi, :],
                                in_=tp[:, 0:(hi - lo) * 128].rearrange(
                                    'p (j d) -> p j d', d=128))
                        # QKV (chunk-outer, j-mid, g-inner: LDW amortized)
                        qf = a_q.tile([128, CPC], bf16, tag='qf')
                        kf = a_q.tile([128, CPC], bf16, tag='kf')
                        vf = a_q.tile([128, CPC], bf16, tag='vf')
                        dsts = (qf, kf, vf)
                        for lo, hi in ((0, 512), (512, 640)):
                            pss = [ps512.tile([128, 512], f32, tag='mm',
                                              name='qkvps')
                                   for _ in range(3)]
                            for j in range(10):
                                for g in range(3):
                                    nc.tensor.matmul(
                                        pss[g][:, 0:hi - lo], xnT[:, j, :],
                                        wq_sb[:, j,
                                              g * CPC + lo:g * CPC + hi],
                                        start=(j == 0), stop=(j == 9))
                            for g in range(3):
                                if g == 2:
                                    nc.vector.tensor_scalar_mul(
                                        out=dsts[g][:, lo:hi],
                                        in0=pss[g][:, 0:hi - lo],
                                        scalar1=rstd[:])
                                else:
                                    nc.scalar.activation(
                                        out=dsts[g][:, lo:hi],
                                        in_=pss[g][:, 0:hi - lo],
                                        func=AF.Copy, scale=rstd[:])
                        stash[t] = (qf, kf, vf)

                    def rope(eng, src, cos_t, sin_t, out):
                        # out[p,h,d] = src*cos + swap(src)*sinneg   (bf16)
                        src3 = src[:].rearrange('p (h d) -> p h d', h=HPC)
                        pa = list(src3.ap)
                        swap = _ap(src3, 64, pa[:2] + [[-64, 2], [1, 64]])
                        ca = list(cos_t.ap)
                        cos4 = _ap(cos_t, 0, [ca[0], [0, HPC], [1, 128]])
                        sin4 = _ap(sin_t, 0,
                                   [ca[0], [0, HPC], [64, 2], [1, 64]])
                        tmp = a_t.tile([128, HPC, D], bf16, tag='rtmp')
                        eng.tensor_tensor(
                            out=tmp[:].rearrange('p h (u d) -> p h u d', u=2),
                            in0=swap, in1=sin4, op=OP.mult)
                        eng.tensor_tensor(out=out[:], in0=src3, in1=cos4,
                                          op=OP.mult)
                        eng.tensor_add(out=out[:], in0=out[:], in1=tmp[:])

                    def blk4(ap20, reps=32):
                        # (128,20) -> (128,5,4,reps) block broadcast
                        a = list(ap20.ap)
                        st = a[-1][0]
                        return bass.AP(tensor=ap20.tensor, offset=ap20.offset,
                                       ap=[a[0], [4 * st, HPC], [st, 4],
                                           [0, reps]])

                    def hb(ap5, reps=4):
                        # (128,5) -> (128,5,reps) broadcast
                        a = list(ap5.ap)
                        return bass.AP(tensor=ap5.tensor, offset=ap5.offset,
                                       ap=[a[0], [a[-1][0], HPC], [0, reps]])

                    def v4(x):
                        return x.rearrange('p h (b e) -> p h b e', e=32)

                    def emit_tail(t):
                        qf, kf, vf = stash.pop(t)
                        # rms of pre-rope q/k (rope is norm-preserving)
                        msq = a_t.tile([128, 2, HPC], f32, tag='msq')
                        for h in range(HPC):
                            nc.scalar.activation(
                                out=scr_sq[:, 0:D],
                                in_=qf[:, h * D:(h + 1) * D],
                                func=AF.Square, bias=zero_sb[:],
                                accum_out=msq[:, 0, h:h + 1])
                        ksq = a_t.tile([128, HPC, D], bf16, tag='ksq')
                        kf3 = kf[:].rearrange('p (h d) -> p h d', h=HPC)
                        nc.vector.tensor_tensor(out=ksq[:], in0=kf3,
                                                in1=kf3, op=OP.mult)
                        nc.vector.tensor_reduce(out=msq[:, 1, :],
                                                in_=ksq[:], axis=AX.X,
                                                op=OP.add)
                        _rsqrt_vec(nc, a_t, msq[:], msq[:],
                                   float(1.0 / D), EPS, 'rqk')
                        # rope (q on vector, k on gpsimd)
                        zq = a_t.tile([128, HPC, D], bf16, tag='zq')
                        rope(nc.vector, qf, cq_sb[:, t, :], sq_sb[:, t, :],
                             zq)
                        zk = a_t.tile([128, HPC, D], bf16, tag='zk')
                        rope(nc.gpsimd, kf, ck_sb[:, t, :], sk_sb[:, t, :],
                             zk)
                        # block amax; amn = amax*rstd (q,k) or amax (v)
                        amn = a_t.tile([128, 3, NBLK], f32, tag='amn')
                        nc.vector.tensor_reduce(
                            out=amn[:, 0, :], in_=v4(zq[:]), axis=AX.X,
                            op=OP.max, apply_absolute_value=True)
                        nc.vector.tensor_reduce(
                            out=amn[:, 1, :], in_=v4(zk[:]), axis=AX.X,
                            op=OP.max, apply_absolute_value=True)
                        nc.vector.tensor_reduce(
                            out=amn[:, 2, :],
                            in_=vf[:].rearrange('p (h b e) -> p h b e',
                                                h=HPC, e=32),
                            axis=AX.X, op=OP.max, apply_absolute_value=True)
                        for i in range(2):
                            nc.vector.tensor_tensor(
                                out=amn[:, i, :].rearrange(
                                    'p (h b) -> p h b', h=HPC),
                                in0=amn[:, i, :].rearrange(
                                    'p (h b) -> p h b', h=HPC),
                                in1=hb(msq[:, i, :]), op=OP.mult)
                        nc.vector.tensor_scalar_max(out=amn[:], in0=amn[:],
                                                    scalar1=1e-12)
                        eb = a_t.tile([128, 3, NBLK], i32, tag='eb')
                        nc.vector.tensor_single_scalar(
                            out=eb[:], in_=amn[:].bitcast(i32), scalar=23,
                            op=OP.logical_shift_right)
                        sc = a_t.tile([128, 3, NBLK], f32, tag='sc')
                        nc.vector.tensor_scalar(
                            out=sc[:].bitcast(i32), in0=eb[:], scalar1=-1,
                            scalar2=260, op0=OP.mult, op1=OP.add)
                        nc.vector.tensor_single_scalar(
                            out=sc[:].bitcast(i32), in_=sc[:].bitcast(i32),
                            scalar=23, op=OP.logical_shift_left)
                        isc = a_t.tile([128, 3, NBLK], f32, tag='isc')
                        nc.vector.tensor_single_scalar(
                            out=isc[:].bitcast(i32), in_=eb[:], scalar=6,
                            op=OP.subtract)
                        nc.vector.tensor_single_scalar(
                            out=isc[:].bitcast(i32), in_=isc[:].bitcast(i32),
                            scalar=23, op=OP.logical_shift_left)
                        msc = a_t.tile([128, 2, NBLK], f32, tag='msc')
                        for i in range(2):
                            nc.vector.tensor_tensor(
                                out=msc[:, i, :].rearrange(
                                    'p (h b) -> p h b', h=HPC),
                                in0=sc[:, i, :].rearrange(
                                    'p (h b) -> p h b', h=HPC),
                                in1=hb(msq[:, i, :]), op=OP.mult)
                        # quantize q (vector)
                        ys = a_t.tile([128, HPC, D], bf16, tag='ys')
                        q8 = a_t.tile([128, HPC, D], fp8, tag='q8')
                        qd = a_t.tile([128, HPC, D], bf16, tag='qd')
                        nc.vector.tensor_tensor(out=v4(ys[:]), in0=v4(zq[:]),
                                                in1=blk4(msc[:, 0, :]),
                                                op=OP.mult)
                        nc.vector.tensor_scalar(out=q8[:], in0=ys[:],
                                                scalar1=-112.0,
                                                scalar2=112.0,
                                                op0=OP.max, op1=OP.min)
                        nc.vector.tensor_tensor(out=v4(qd[:]), in0=v4(q8[:]),
                                                in1=blk4(isc[:, 0, :]),
                                                op=OP.mult)
                        # quantize k (gpsimd mults, vector fp8 cast)
                        ysk = a_t.tile([128, HPC, D], bf16, tag='ys')
                        k8 = a_t.tile([128, HPC, D], fp8, tag='q8')
                        kd = a_t.tile([128, HPC, D], bf16, tag='kd')
                        nc.gpsimd.tensor_tensor(out=v4(ysk[:]),
                                                in0=v4(zk[:]),
                                                in1=blk4(msc[:, 1, :]),
                                                op=OP.mult)
                        nc.vector.tensor_scalar(out=k8[:], in0=ysk[:],
                                                scalar1=-112.0,
                                                scalar2=112.0,
                                                op0=OP.max, op1=OP.min)
                        nc.gpsimd.tensor_tensor(out=v4(kd[:]), in0=v4(k8[:]),
                                                in1=blk4(isc[:, 1, :]),
                                                op=OP.mult)
                        # quantize v (vector; deq straight into vd_sb)
                        ysv = a_t.tile([128, HPC, D], bf16, tag='ys')
                        v8 = a_t.tile([128, HPC, D], fp8, tag='q8')
                        nc.vector.tensor_tensor(
                            out=v4(ysv[:]),
                            in0=v4(vf[:].rearrange('p (h d) -> p h d',
                                                   h=HPC)),
                            in1=blk4(sc[:, 2, :]), op=OP.mult)
                        nc.vector.tensor_scalar(out=v8[:], in0=ysv[:],
                                                scalar1=-112.0,
                                                scalar2=112.0,
                                                op0=OP.max, op1=OP.min)
                        nc.vector.tensor_tensor(
                            out=v4(vd_sb[:, t, :, 0:D]), in0=v4(v8[:]),
                            in1=blk4(isc[:, 2, :]), op=OP.mult)
                        # transpose qd/kd into qT/kT
                        for src, dstT in ((qd, qT), (kd, kT)):
                            tp = psT.tile([128, 640], bf16, tag='tp')
                            for h in range(HPC):
                                nc.tensor.transpose(
                                    tp[:, h * 128:(h + 1) * 128],
                                    src[:, h, :], ident_b[:])
                            nc.vector.tensor_copy(
                                out=dstT[:, :, t * 128:(t + 1) * 128],
                                in_=tp[:].rearrange('p (h d) -> p h d',
                                                    h=HPC))

                    def emit_attn(qb):
                        nkt = 4 * qb + 4
                        for h in range(HPC):
                            dps = psD.tile([128, 512], f32, tag='dps')
                            ops = ops_ps.tile([128, 512], f32, tag='ops')
                            for kt in range(nkt):
                                sp = ps512.tile([128, 512], f32, tag='mm')
                                o = kt - 4 * qb
                                nc.tensor.matmul(
                                    sp[:],
                                    kT[:, h, kt * 128:(kt + 1) * 128],
                                    qT[:, h, qb * 512:(qb + 1) * 512],
                                    start=True, stop=True)
                                pT = pT_pool.tile([128, 512], bf16, tag='pT')
                                nc.scalar.activation(out=pT[:], in_=sp[:],
                                                     func=AF.Exp,
                                                     bias=zero_sb[:],
                                                     scale=INV_SQRT_D)
                                if o >= 0:
                                    nc.gpsimd.affine_select(
                                        out=pT[:], in_=pT[:],
                                        compare_op=OP.is_ge, fill=0.0,
                                        base=-128 * o, pattern=[[1, 512]],
                                        channel_multiplier=-1)
                                nc.tensor.matmul(dps[:], ones128[:], pT[:],
                                                 start=(kt == 0),
                                                 stop=(kt == nkt - 1))
                                nc.tensor.matmul(ops[:],
                                                 vd_sb[:, kt, h, 0:128],
                                                 pT[:],
                                                 start=(kt == 0),
                                                 stop=(kt == nkt - 1))
                            rd = b_t.tile([128, 512], f32, tag='rd')
                            nc.vector.reciprocal_approx_fast(out=rd[:],
                                                             in_=dps[:])
                            nc.vector.tensor_tensor(
                                out=attnT[:, h, qb * 512:(qb + 1) * 512],
                                in0=ops[:], in1=rd[:], op=OP.mult)

                    wo_sb = a_w.tile([128, HPC, C], bf16)
                    nc.sync.dma_start(
                        out=wo_sb[:],
                        in_=wout_d.ap().rearrange('(h p) c -> p h c', p=128))
                    # rs_in row layout: [t0:512 | t1024:1536 | t512:1024
                    # | t1536:2048] so each RS half is contiguous.
                    rowblk = {tt: i for i, tt in enumerate(
                        (0, 1, 2, 3, 8, 9, 10, 11, 4, 5, 6, 7,
                         12, 13, 14, 15))}
                    grp = [[2 * i, 2 * i + 1] for i in range(n_cores // 2)]

                    def oproj(tt):
                        ob = b_t.tile([128, C], bf16, tag='ob', name='ob')
                        for ci, (lo, hi) in enumerate(((0, 512),
                                                       (512, 1024),
                                                       (1024, C))):
                            ps = ps512.tile([128, 512], f32, tag='mm',
                                            name='oprojps')
                            for h in range(HPC):
                                nc.tensor.matmul(
                                    ps[:, 0:hi - lo],
                                    attnT[:, h, tt * 128:(tt + 1) * 128],
                                    wo_sb[:, h, lo:hi],
                                    start=(h == 0), stop=(h == HPC - 1))
                            if ci == 2:
                                nc.scalar.copy(out=ob[:, lo:hi],
                                               in_=ps[:, 0:hi - lo])
                            else:
                                nc.vector.tensor_copy(
                                    out=ob[:, lo:hi], in_=ps[:, 0:hi - lo])
                        r = rowblk[tt]
                        nc.sync.dma_start(
                            out=rs_in[r * 128:(r + 1) * 128, :], in_=ob[:])

                    # ---- interleaved A+B+C emission ----
                    for t in range(NT):
                        emit_head(t)
                        if t >= 1:
                            emit_tail(t - 1)
                        if t >= 4 and t % 4 == 0:
                            qb = t // 4 - 1
                            emit_attn(qb)
                            for tt in range(4 * qb, 4 * qb + 4):
                                oproj(tt)
                            if qb == 2:
                                nc.gpsimd.collective_compute(
                                    'ReduceScatter', OP.add,
                                    ins=[rs_in[0:1024, :].opt()],
                                    outs=[rs_out[0:512, :].opt()],
                                    replica_groups=grp)
                    emit_tail(NT - 1)
                    emit_attn(QB - 1)
                    for tt in range(12, 16):
                        oproj(tt)
                    nc.gpsimd.collective_compute(
                        'ReduceScatter', OP.add,
                        ins=[rs_in[1024:2048, :].opt()],
                        outs=[rs_out[512:1024, :].opt()],
                        replica_groups=grp)

            # ====== phase D: residual + MLP over my T/2 tokens ======
            with contextlib.ExitStack() as pd:
                d_t = pd.enter_context(tc.tile_pool(name='d_t', bufs=2))
                d_big = pd.enter_context(tc.tile_pool(name='d_big', bufs=1))
                x2_sb = d_big.tile([128, NH, C], f32)
                xn2T = d_big.tile([128, 10, T // 2], bf16)
                h2T = d_big.tile([128, F // 128, T // 2], bf16)
                rinv_sb = d_big.tile([128, NH], f32)

                with tc.tile_pool(name='d_ps', bufs=4, space='PSUM') as d_ps, \
                     tc.tile_pool(name='dt_ps', bufs=2,
                                  space='PSUM') as dt_ps:
                    for tt in range(NH):
                        rsx = d_t.tile([128, C], bf16, tag='rsx')
                        nc.gpsimd.dma_start(
                            out=rsx[:],
                            in_=rs_out[tt * 128:(tt + 1) * 128, :])
                        xht = d_t.tile([128, C], f32, tag='xht')
                        nc.sync.dma_start(
                            out=xht[:],
                            in_=xh_d[tt * 128:(tt + 1) * 128, :])
                        nc.vector.tensor_add(out=x2_sb[:, tt, :],
                                             in0=rsx[:], in1=xht[:])
                        ssq2 = d_t.tile([128, 1], f32, tag='ssq2')
                        nc.scalar.activation(out=scr_sq[:],
                                             in_=x2_sb[:, tt, :],
                                             func=AF.Square, bias=zero_sb[:],
                                             accum_out=ssq2[:])
                        m2 = d_t.tile([128, 1], f32, tag='m2')
                        nc.vector.tensor_scalar(out=m2[:], in0=ssq2[:],
                                                scalar1=float(1.0 / C),
                                                scalar2=EPS,
                                                op0=OP.mult, op1=OP.add)
                        nc.vector.reciprocal_approx_fast(
                            out=rinv_sb[:, tt:tt + 1], in_=m2[:])
                        for jg, (lo, hi) in enumerate(((0, 4), (4, 8),
                                                      (8, 10))):
                            tp2 = dt_ps.tile([128, 512], f32, tag='tp2')
                            for j in range(lo, hi):
                                nc.tensor.transpose(
                                    tp2[:, (j - lo) * 128:(j - lo + 1) * 128],
                                    x2_sb[:, tt, j * 128:(j + 1) * 128],
                                    ident_f[:])
                            nc.vector.tensor_copy(
                                out=xn2T[:, lo:hi, tt * 128:(tt + 1) * 128],
                                in_=tp2[:, 0:(hi - lo) * 128].rearrange(
                                    'p (j d) -> p j d', d=128))

                    # fc1: j-loop with LDW amortized over two 512 chunks
                    with tc.tile_pool(name='wf1', bufs=5) as wf1_pool:
                        for fi in range(F // 128):
                            wf1 = wf1_pool.tile([128, 10, 128], bf16,
                                                tag='wf1')
                            nc.sync.dma_start(
                                out=wf1[:],
                                in_=wfc1_d[:, fi * 128:(fi + 1) * 128]
                                .rearrange('(j p) c -> p j c', p=128))
                            hp0 = d_ps.tile([128, 512], f32, tag='hps')
                            hp1 = d_ps.tile([128, 512], f32, tag='hps')
                            for j in range(10):
                                nc.tensor.matmul(hp0[:], wf1[:, j, :],
                                                 xn2T[:, j, 0:512],
                                                 start=(j == 0),
                                                 stop=(j == 9))
                                nc.tensor.matmul(hp1[:], wf1[:, j, :],
                                                 xn2T[:, j, 512:1024],
                                                 start=(j == 0),
                                                 stop=(j == 9))
                            for ci, hp in ((0, hp0), (1, hp1)):
                                hrelu = d_t.tile([128, 512], bf16,
                                                 tag='hrelu')
                                nc.scalar.activation(out=hrelu[:], in_=hp[:],
                                                     func=AF.Relu,
                                                     bias=zero_sb[:])
                                nc.vector.tensor_mul(
                                    out=h2T[:, fi,
                                            ci * 512:(ci + 1) * 512],
                                    in0=hrelu[:], in1=hrelu[:])

                # fc2: c-halves x tt-quads; 2 matmuls (640 cols) per lhsT
                with tc.tile_pool(name='y_ps', bufs=4, space='PSUM') as y_ps, \
                     tc.tile_pool(name='wf2', bufs=3) as wf2_pool:
                    for clo, chi in ((0, 640), (640, C)):
                        for ttg in range(2):
                            yps = [y_ps.tile([128, 640], f32, tag='yps',
                                             name='yps')
                                   for _ in range(4)]
                            for f2 in range(F // 256):
                                wf2 = wf2_pool.tile([128, 2, 640], bf16,
                                                    tag='wf2')
                                nc.sync.dma_start(
                                    out=wf2[:],
                                    in_=wfc2_d[f2 * 256:(f2 + 1) * 256,
                                               clo:chi]
                                    .rearrange('(u p) c -> p u c', p=128))
                                for u in range(2):
                                    fi = 2 * f2 + u
                                    st = (fi == 0)
                                    sp_ = (fi == F // 128 - 1)
                                    for i in range(4):
                                        tt = 4 * ttg + i
                                        lhsT = h2T[:, fi,
                                                   tt * 128:(tt + 1) * 128]
                                        nc.tensor.matmul(
                                            yps[i][:, 0:512], lhsT,
                                            wf2[:, u, 0:512],
                                            start=st, stop=sp_)
                                        nc.tensor.matmul(
                                            yps[i][:, 512:640], lhsT,
                                            wf2[:, u, 512:640],
                                            start=st, stop=sp_)
                            for i in range(4):
                                tt = 4 * ttg + i
                                yo = d_t.tile([128, 640], f32, tag='yo')
                                nc.vector.scalar_tensor_tensor(
                                    out=yo[:], in0=yps[i][:],
                                    scalar=rinv_sb[:, tt:tt + 1],
                                    in1=x2_sb[:, tt, clo:chi],
                                    op0=OP.mult, op1=OP.add)
                                nc.sync.dma_start(
                                    out=y_d[tt * 128:(tt + 1) * 128,
                                            clo:chi],
                                    in_=yo[:])

    nc.compile()
    return nc


_CACHE = {}


def _get_nc(t_len=T):
    if t_len not in _CACHE:
        _CACHE[t_len] = build_nc(t_len)
    return _CACHE[t_len]


def make_in_maps(x, rotary_pos_emb, ln1_w, w_qkv, qn_w, kn_w, w_out, ln2_w,
                 w_fc1, w_fc2, t_len=T):
    """Host-side sharding prep. Returns list of per-core input dicts."""
    x = np.asarray(x, np.float32)
    rot = np.asarray(rotary_pos_emb, np.float32)
    cos = np.cos(rot).astype(np.float32)
    sin = np.sin(rot).astype(np.float32)
    sinneg = np.concatenate([-sin[:, :64], sin[:, :64]], axis=-1)
    qn = np.asarray(qn_w, np.float32)
    kn = np.asarray(kn_w, np.float32)
    cosq = (cos * qn).astype(ml_dtypes.bfloat16)
    sinq = (sinneg * qn).astype(ml_dtypes.bfloat16)
    cosk = (cos * kn).astype(ml_dtypes.bfloat16)
    sink = (sinneg * kn).astype(ml_dtypes.bfloat16)
    w_qkv_f = (np.asarray(w_qkv, np.float32)
               * np.asarray(ln1_w, np.float32)[:, None]).reshape(C, 3, H, D)
    w_fc1_f = (np.asarray(w_fc1, np.float32)
               * np.asarray(ln2_w, np.float32)[:, None]
               ).astype(ml_dtypes.bfloat16)
    w_fc2_b = np.asarray(w_fc2, np.float32).astype(ml_dtypes.bfloat16)
    wo = np.asarray(w_out, np.float32).reshape(H, D, C)

    in_maps = []
    for c in range(N_CORES):
        b, hg = c // 2, c % 2
        heads = slice(hg * HPC, (hg + 1) * HPC)
        wq = np.ascontiguousarray(
            w_qkv_f[:, :, heads, :].reshape(C, 3 * CPC)
        ).astype(ml_dtypes.bfloat16)
        w_outp = np.ascontiguousarray(
            wo[heads].reshape(CPC, C)).astype(ml_dtypes.bfloat16)
        in_maps.append({
            'x': np.ascontiguousarray(x[b]),
            'xh': np.ascontiguousarray(x[b, hg * T // 2:(hg + 1) * T // 2]),
            'w_qkv': wq,
            'cosq': cosq, 'sinq': sinq, 'cosk': cosk, 'sink': sink,
            'w_out': w_outp,
            'w_fc1': np.ascontiguousarray(w_fc1_f),
            'w_fc2': np.ascontiguousarray(w_fc2_b),
        })
    return in_maps


def assemble_output(results, t_len=T):
    out = np.zeros((B, t_len, C), np.float32)
    for c in range(N_CORES):
        b, hg = c // 2, c % 2
        out[b, hg * t_len // 2:(hg + 1) * t_len // 2] = results[c]['y']
    return out


def kernel(**inputs):
    nc = _get_nc(T)
    in_maps = make_in_maps(**inputs)
    res = bass_utils.run_bass_kernel_spmd(nc, in_maps,
                                          core_ids=list(range(N_CORES)))
    return assemble_output(res.results)
